# revision 1
# baseline (speedup 1.0000x reference)
"""Cosformer causal attention (B=1, L=2048, E=512, H=8) on 8 TRN2 NeuronCores.

One head per core; single raw-x projection streams; trig folded into
constants wherever the cosformer algebra allows:
  A[l, m] = relu(q_l).relu(k_m) * cos(th_l - th_m)   (th = pi/2 * pos/L)

Per core (head h), from raw xT [e_in, L] (bf16):
  T-stream : psum = [Wq|Wk]^T xT -> relu -> qk [128, L] (q rows 0-63, k 64-127)
  scalings : qcqs = [q'*cos ; q'*sin] via 2 DVE ops + DMA partition shift;
             kraw = k' shifted to partitions 0-63 (DMA); no scaled k needed --
             the intra-chunk cos(th_l - th_m) is TOEPLITZ, folded into the
             causal mask constant (same [C, C] matrix for every chunk).
  N-stream : per chunk psum = xT_chunk^T [Wk|Wv] -> krelu, v natural;
             kn = [k'*cos | k'*sin] (free-dim packed, gpsimd), vaug = [v, 1]
  P2 (C=128 chunks): s0 = kraw^T q' ; at = s0 * maskcos (DVE);
             O = at^T vaug + qcqs^T kvsnap (K=128 packs cos+sin branches);
             KV state accumulates in a persistent PSUM bank (kn^T vaug, one
             M=128-packed matmul/chunk) with per-chunk bf16 snapshots; the
             state chain is emitted ahead of each group so its latency
             overlaps the S0/intra work.
  P3: obf = O[:, :64]/(O[:, 64]+eps); PAIRED PE transposes (2 chunks ->
      [128, C], even chunk rows 0-63 / odd rows 64-127) -> otb [128, 8, C];
      two-row-stream out-projection against w2dup (W_out head rows duplicated
      to both partition halves); outP columns are chunk-permuted
      (CHUNK_OF_SLOT; host unpermutes).
Emission is software-pipelined: body i+1's DMAs/projections are interleaved
into body i's P2 groups (in-order engines fill dependency stalls).
Host: out = sum_h unperm(outP_h)^T + b_out.  rel err ~4.6e-3 vs reference.
"""

import numpy as np
import ml_dtypes

import concourse.bass as bass
import concourse.mybir as mybir
from concourse.tile import TileContext
from concourse.vector_clock import ScopedClock

BF16 = mybir.dt.bfloat16
F32 = mybir.dt.float32
AF = mybir.ActivationFunctionType
ALU = mybir.AluOpType

B, L, E, H = 1, 2048, 512, 8
D = E // H            # 64 head dim
C = 128               # chunk length
NCH = L // C          # 16 chunks
GRP = 4               # chunks per psum group
NG = NCH // GRP       # 4 groups
LT = 512              # l tile for N=512 matmul streams
NLT = L // LT         # 4
EPS = 1e-6
N_CORES = 8

# outP column slot s holds chunk CHUNK_OF_SLOT[s] (see P3 pairing)
CHUNK_OF_SLOT = [0, 2, 4, 6, 1, 3, 5, 7, 8, 10, 12, 14, 9, 11, 13, 15]


def _split_multi_waits(bir_json):
    """The walrus in this container accepts at most ONE sem wait per
    instruction; split extras into standalone EventSemaphore waits placed
    immediately before the instruction (same engine => order preserved)."""
    import json as _json

    js = _json.loads(bir_json)
    ctr = 0
    for fn in js.get("functions", []):
        for bb in fn.get("blocks", []):
            insts = bb.get("instructions")
            if not insts:
                continue
            out = []
            changed = False
            for inst in insts:
                si = inst.get("sync_info")
                waits = si.get("on_wait", []) if si else []
                if len(waits) > 1:
                    changed = True
                    for w in waits[:-1]:
                        ctr += 1
                        out.append({
                            "debug": inst.get("debug", 0),
                            "engine": inst["engine"],
                            "ins": [],
                            "name": f"I-splitw-{ctr}",
                            "opcode": "EventSemaphore",
                            "outs": [],
                            "sync_info": {"on_update": [], "on_wait": [w]},
                        })
                    si["on_wait"] = [waits[-1]]
                out.append(inst)
            if changed:
                bb["instructions"] = out
    return _json.dumps(js).encode()


def _install_wait_split_hook():
    import concourse.bass2jax as bass2jax
    import concourse.bass_utils as bass_utils

    if getattr(bass2jax, "_wait_split_installed", False):
        return
    orig = bass_utils.compile_bir_kernel

    def patched(bir_json, tmpdir, neff_name="file.neff"):
        return orig(_split_multi_waits(bir_json), tmpdir, neff_name=neff_name)

    bass2jax.compile_bir_kernel = patched
    bass_utils.compile_bir_kernel = patched
    bass2jax._wait_split_installed = True


_install_wait_split_hook()


class SplitDrainTileContext(TileContext):
    """walrus in this container rejects >1 sem wait on the final SP Drain;
    spread the accumulated waits over single-wait SP wait instructions."""

    def _drain_and_barrier(self, tick_clock, wait_clock):
        nc = self.nc
        drain_inst = nc.sync.drain()
        wait_clock.add_sem_waits(
            drain_inst.ins, ScopedClock({None: tick_clock.global_clock})
        )
        waits = list(drain_inst.ins.sync_info.on_wait)
        if len(waits) > 1:
            drain_inst.ins.sync_info.on_wait = waits[:1]
            name2sem = {v.name: v for v in self.sems.allocated().values()}
            for w in waits[1:]:
                nc.sync.wait_ge(name2sem[w.ant_name], w.wait_value)
        nc.all_engine_barrier()
        popped = nc._tile_sem_poison_stack.pop()
        assert popped is self._sem_poison
        nc.clear_and_free_semaphores(list(self.sems.allocated().values()))
        nc.all_engine_barrier()


def build_program(e_in=E, repeat=1, debug=False):
    nc = bass.Bass("TRN2", target_bir_lowering=False, debug=False,
                   num_devices=N_CORES)

    ecs = [(i, 128) for i in range(4)]
    if e_in > 4 * 128:
        assert e_in == 4 * 128 + 1
        ecs.append((4, e_in - 4 * 128))
    NEC = len(ecs)

    xT = nc.dram_tensor("xT", [e_in, L], BF16, kind="ExternalInput")
    wqk = nc.dram_tensor("wqk", [e_in, 128], BF16, kind="ExternalInput")
    wkv = nc.dram_tensor("wkv", [e_in, 128], BF16, kind="ExternalInput")
    w2d = nc.dram_tensor("w2d", [128, E], BF16, kind="ExternalInput")
    cosTd = nc.dram_tensor("cosT", [1, L], BF16, kind="ExternalInput")
    sinTd = nc.dram_tensor("sinT", [1, L], BF16, kind="ExternalInput")
    cosNd = nc.dram_tensor("cosN", [C, NCH], BF16, kind="ExternalInput")
    sinNd = nc.dram_tensor("sinN", [C, NCH], BF16, kind="ExternalInput")
    maskd = nc.dram_tensor("mask", [C, C], BF16, kind="ExternalInput")
    identd = nc.dram_tensor("ident", [C, C], BF16, kind="ExternalInput")
    outP = nc.dram_tensor("outP", [E, L], BF16, kind="ExternalOutput")

    with SplitDrainTileContext(nc) as tc:
        with (
            tc.tile_pool(name="const", bufs=1) as cpool,
            tc.tile_pool(name="work", bufs=2) as wpool,
            tc.tile_pool(name="stage", bufs=3) as spool,
            tc.tile_pool(name="pbig", bufs=2, space="PSUM") as pbig,
            tc.tile_pool(name="pso", bufs=2, space="PSUM") as pso,
            tc.tile_pool(name="pkv", bufs=2, space="PSUM") as pkv,
        ):
            # ---- constants (one slot, DMA'd once) ----
            wqk_sb = cpool.tile([128, NEC, 128], BF16, tag="wqk")
            wkv_sb = cpool.tile([128, NEC, 128], BF16, tag="wkv")
            w2d_sb = cpool.tile([128, NLT, 128], BF16, tag="w2d")
            cosT_sb = cpool.tile([128, L], BF16, tag="cosT")
            sinT_sb = cpool.tile([128, L], BF16, tag="sinT")
            cosN_sb = cpool.tile([C, NCH], BF16, tag="cosN")
            sinN_sb = cpool.tile([C, NCH], BF16, tag="sinN")
            mask_sb = cpool.tile([C, C], BF16, tag="mask")
            ident_sb = cpool.tile([C, C], BF16, tag="ident")

            for t_sb, t_d in [(wqk_sb, wqk), (wkv_sb, wkv)]:
                nc.sync.dma_start(
                    t_sb[:, :4, :],
                    t_d[: 4 * 128, :].rearrange("(c p) d -> p c d", p=128),
                )
                if NEC == 5:
                    nc.sync.dma_start(t_sb[:1, 4, :], t_d[4 * 128 :, :])
            nc.sync.dma_start(
                cosT_sb[:], cosTd[:1, :].to_broadcast([128, L]))
            nc.sync.dma_start(
                sinT_sb[:], sinTd[:1, :].to_broadcast([128, L]))
            nc.sync.dma_start(
                w2d_sb[:], w2d.rearrange("d (t n) -> d t n", n=128)
            )
            nc.sync.dma_start(cosN_sb[:], cosNd[:])
            nc.sync.dma_start(sinN_sb[:], sinNd[:])
            nc.sync.dma_start(mask_sb[:], maskd[:])
            nc.sync.dma_start(ident_sb[:], identd[:])

            def make_tiles():
                tl = {}
                tl["xT"] = wpool.tile([128, NEC, L], BF16, tag="xT", name="xTs")
                tl["qk"] = wpool.tile([128, L], BF16, tag="qk", name="qkt")
                tl["qcqs"] = wpool.tile([128, L], BF16, tag="qcqs", name="qcqst")
                tl["kraw"] = wpool.tile([D, L], BF16, tag="kraw", name="krawt")
                tl["qstmp"] = wpool.tile([D, L], BF16, tag="qstmp", name="qstmpt")
                tl["krelu"] = wpool.tile([C, NCH, D], BF16, tag="krelu", name="krelut")
                tl["kn"] = wpool.tile([C, NCH, 128], BF16, tag="kn", name="knt")
                tl["vaug"] = wpool.tile([C, NCH, D + 1], BF16, tag="vaug", name="vaugt")
                tl["at"] = wpool.tile([C, NCH, C], BF16, tag="at", name="att")
                tl["kvs"] = wpool.tile([128, NCH - 1, D + 1], BF16, tag="kvs", name="kvst")
                tl["obf"] = wpool.tile([C, NCH, D], BF16, tag="obf", name="obft")
                tl["otb"] = wpool.tile([128, NCH // 2, C], BF16, tag="otb", name="otbt")
                tl["rtmp"] = wpool.tile([C, NCH], F32, tag="rtmp", name="rtmpt")
                tl["r"] = wpool.tile([C, NCH], F32, tag="r", name="rt")
                return tl

            def a_parts(tl):
                """Input DMA + projections + scalings for one body, as a list
                of emission thunks (interleaved into the previous body)."""
                parts = []

                def dma_x(t, eng):
                    def f():
                        ls = slice(t * LT, (t + 1) * LT)
                        eng.dma_start(
                            tl["xT"][:, :4, ls],
                            xT[: 4 * 128, ls].rearrange(
                                "(c p) l -> p c l", p=128),
                        )
                        if NEC == 5 and t == 0:
                            eng.dma_start(tl["xT"][:1, 4, :], xT[4 * 128 :, :])
                        if t == 0:
                            nc.gpsimd.memset(tl["vaug"][:, :, D : D + 1], 1.0)
                    return f

                def tproj(tp):
                    def f():
                        pT = pbig.tile([128, 2, LT], F32, tag="big",
                                       name="pT")
                        for half in range(2):
                            t = 2 * tp + half
                            ls = slice(t * LT, (t + 1) * LT)
                            for i, (ec, pc) in enumerate(ecs):
                                nc.tensor.matmul(
                                    pT[:, half, :], wqk_sb[:pc, ec, :],
                                    tl["xT"][:pc, ec, ls],
                                    start=(i == 0), stop=(i == NEC - 1),
                                )
                        ls2 = slice(tp * 2 * LT, (tp + 1) * 2 * LT)
                        if tp == 0:
                            nc.scalar.activation(tl["qk"][:, ls2], pT[:],
                                                 AF.Relu)
                        else:
                            nc.vector.tensor_scalar_max(tl["qk"][:, ls2],
                                                        pT[:], 0.0)
                    return f

                def scal(hl):
                    def f():
                        ls = slice(hl * (L // 2), (hl + 1) * (L // 2))
                        nc.vector.tensor_tensor(
                            tl["qcqs"][:D, ls], tl["qk"][:D, ls],
                            cosT_sb[:D, ls], ALU.mult,
                        )
                        nc.vector.tensor_tensor(
                            tl["qstmp"][:, ls], tl["qk"][:D, ls],
                            sinT_sb[:D, ls], ALU.mult,
                        )
                        for q in range(2):
                            qs_ = slice((2 * hl + q) * LT,
                                        (2 * hl + q + 1) * LT)
                            nc.sync.dma_start(tl["kraw"][:, qs_],
                                              tl["qk"][D:, qs_])
                            nc.scalar.dma_start(tl["qcqs"][D:, qs_],
                                                tl["qstmp"][:, qs_])
                    return f

                def nstream(h):
                    def f():
                        pN = pbig.tile([C, 2 * GRP, 128], F32, tag="big",
                                       name="pN")
                        for sub in range(2 * GRP):
                            j = h * 2 * GRP + sub
                            cs = slice(j * C, (j + 1) * C)
                            for i, (ec, pc) in enumerate(ecs):
                                nc.tensor.matmul(
                                    pN[:, sub, :], tl["xT"][:pc, ec, cs],
                                    wkv_sb[:pc, ec, :],
                                    start=(i == 0), stop=(i == NEC - 1),
                                )
                        gs = slice(h * 2 * GRP, (h + 1) * 2 * GRP)
                        nc.scalar.activation(
                            tl["krelu"][:, gs, :], pN[:, :, :D], AF.Relu)
                        nc.scalar.activation(
                            tl["vaug"][:, gs, :D], pN[:, :, D:], AF.Copy)
                        nc.gpsimd.tensor_tensor(
                            tl["kn"][:, gs, :D], tl["krelu"][:, gs, :],
                            cosN_sb[:, gs, None].to_broadcast(
                                [C, 2 * GRP, D]),
                            ALU.mult,
                        )
                        nc.gpsimd.tensor_tensor(
                            tl["kn"][:, gs, D:], tl["krelu"][:, gs, :],
                            sinN_sb[:, gs, None].to_broadcast(
                                [C, 2 * GRP, D]),
                            ALU.mult,
                        )
                    return f

                parts.append(dma_x(0, nc.gpsimd))
                parts.append(dma_x(1, nc.gpsimd))
                parts.append(dma_x(2, nc.gpsimd))
                parts.append(dma_x(3, nc.gpsimd))
                parts.append(tproj(0))
                parts.append(scal(0))
                parts.append(tproj(1))
                parts.append(scal(1))
                parts.append(nstream(0))
                parts.append(nstream(1))
                return parts

            def state_steps(tl, kv_ps, g):
                # kv accumulation + snapshots for chunks of group g (emitted
                # ahead of the group's O work; chain overlaps S0/intra phase)
                for sub in range(GRP):
                    j = g * GRP + sub
                    if j < NCH - 1:
                        nc.tensor.matmul(
                            kv_ps[:, : D + 1], tl["kn"][:, j, :],
                            tl["vaug"][:, j, :],
                            start=(j == 0), stop=(j == NCH - 2),
                            skip_group_check=True,
                        )
                        if j % 2 == 0:
                            nc.scalar.activation(
                                tl["kvs"][:, j, :], kv_ps[:, : D + 1],
                                AF.Copy)
                        else:
                            nc.vector.tensor_copy(
                                tl["kvs"][:, j, :], kv_ps[:, : D + 1])

            def b_group(tl, g):
                gs = slice(g * GRP, (g + 1) * GRP)
                s0 = pso.tile([C, GRP, C], F32, tag="so")
                for sub in range(GRP):
                    j = g * GRP + sub
                    cs = slice(j * C, (j + 1) * C)
                    nc.tensor.matmul(
                        s0[:, sub, :], tl["kraw"][:, cs], tl["qk"][:D, cs],
                        start=True, stop=True,
                    )
                nc.vector.tensor_tensor(
                    tl["at"][:, gs, :], s0[:],
                    mask_sb[:, None, :].to_broadcast([C, GRP, C]), ALU.mult,
                )
                o_ps = pso.tile([C, GRP, 128], F32, tag="so")
                for sub in range(GRP):
                    j = g * GRP + sub
                    cs = slice(j * C, (j + 1) * C)
                    nc.tensor.matmul(
                        o_ps[:, sub, : D + 1], tl["at"][:, j, :],
                        tl["vaug"][:, j, :],
                        start=True, stop=(j == 0),
                    )
                    if j > 0:
                        nc.tensor.matmul(
                            o_ps[:, sub, : D + 1], tl["qcqs"][:, cs],
                            tl["kvs"][:, j - 1, :],
                            start=False, stop=True,
                        )
                nc.vector.tensor_scalar_add(
                    tl["rtmp"][:, gs], o_ps[:, :, D], EPS)
                nc.vector.reciprocal(tl["r"][:, gs], tl["rtmp"][:, gs])
                nc.vector.tensor_tensor(
                    tl["obf"][:, gs, :], o_ps[:, :, :D],
                    tl["r"][:, gs, None].to_broadcast([C, GRP, D]), ALU.mult,
                )

            def c_tr(tl, h2):
                tp = pso.tile([128, NCH // 4, C], BF16, tag="so", name="tp")
                for tt in range(NCH // 4):
                    t = h2 * (NCH // 4) + tt
                    nc.tensor.transpose(
                        tp[:, tt, :], tl["obf"][:, 2 * t : 2 * t + 2, :],
                        ident_sb[:],
                    )
                ts = slice(h2 * 4, (h2 + 1) * 4)
                nc.vector.tensor_copy(tl["otb"][:, ts, :], tp[:])

            def c_p3(tl, h2, nss):
                ts = slice(h2 * 4, (h2 + 1) * 4)
                for ns in nss:
                    pab = pbig.tile([128, 2, LT], F32, tag="big", name="pab")
                    nc.tensor.matmul(
                        pab[:, 0, :], w2d_sb[:D, ns, :], tl["otb"][:D, ts, :],
                        start=True, stop=True,
                    )
                    nc.tensor.matmul(
                        pab[:, 1, :], w2d_sb[D:, ns, :], tl["otb"][D:, ts, :],
                        start=True, stop=True,
                    )
                    ob = spool.tile([128, 2, LT], BF16, tag="ob")
                    if (h2 * NLT + ns) % 4 == 1:
                        nc.vector.tensor_copy(ob[:], pab[:])
                    else:
                        nc.scalar.activation(ob[:], pab[:], AF.Copy)
                    nc.sync.dma_start(
                        outP[ns * 128 : (ns + 1) * 128,
                             h2 * 1024 : (h2 + 1) * 1024],
                        ob[:],
                    )

            # ---- software-pipelined emission across bodies ----
            cur = make_tiles()
            carry = []
            for p in a_parts(cur):
                p()
            for rep in range(repeat):
                if rep + 1 < repeat:
                    nxt = make_tiles()
                    pending = a_parts(nxt)
                else:
                    nxt, pending = None, []
                pi = iter(pending)

                def drain(n):
                    for _ in range(n):
                        p = next(pi, None)
                        if p is not None:
                            p()

                kv_ps = pkv.tile([128, D + 1], F32, tag="kv")
                for g in range(NG):
                    state_steps(cur, kv_ps, g)
                    b_group(cur, g)
                    if g == 1:
                        c_tr(cur, 0)
                        c_p3(cur, 0, [0, 1, 2, 3])
                    drain(3 if g < 2 else 2)
                c_tr(cur, 1)
                c_p3(cur, 1, [0, 1, 2, 3])
                for p in pi:
                    p()
                cur = nxt

    return nc


def prepare_in_maps(x, W_qkv, b_qkv, W_out):
    """Host-side sharding/layout prep. Returns (in_maps, e_in)."""
    x = np.asarray(x, dtype=np.float32).reshape(L, E)
    W_qkv = np.asarray(W_qkv, dtype=np.float32)
    b_qkv = np.asarray(b_qkv, dtype=np.float32)
    W_out = np.asarray(W_out, dtype=np.float32)

    use_bias = bool(np.any(b_qkv))
    if use_bias:
        x_aug = np.concatenate([x, np.ones((L, 1), np.float32)], axis=1)
        W_aug = np.concatenate([W_qkv, b_qkv[None, :]], axis=0)
    else:
        x_aug, W_aug = x, W_qkv
    e_in = x_aug.shape[1]

    bf = ml_dtypes.bfloat16
    xT = np.ascontiguousarray(x_aug.T).astype(bf)

    pos = np.arange(L, dtype=np.float32)
    theta = (np.pi / 2) * pos / L
    cosw = np.cos(theta).astype(np.float32)
    sinw = np.sin(theta).astype(np.float32)
    cosT = np.ascontiguousarray(cosw[None, :]).astype(bf)
    sinT = np.ascontiguousarray(sinw[None, :]).astype(bf)
    cosN = np.ascontiguousarray(cosw.reshape(NCH, C).T).astype(bf)
    sinN = np.ascontiguousarray(sinw.reshape(NCH, C).T).astype(bf)
    # mask[m, l] = cos(theta_l - theta_m) for m <= l else 0  (Toeplitz intra)
    lm = np.arange(C, dtype=np.float32)
    dth = (np.pi / 2) * (lm[None, :] - lm[:, None]) / L
    mask = (np.triu(np.ones((C, C), np.float32)) * np.cos(dth)).astype(bf)
    ident = np.eye(C, dtype=np.float32).astype(bf)

    in_maps = []
    for h in range(N_CORES):
        hs = slice(h * D, (h + 1) * D)
        wq_h = W_aug[:, hs]
        wk_h = W_aug[:, E + h * D : E + (h + 1) * D]
        wv_h = W_aug[:, 2 * E + h * D : 2 * E + (h + 1) * D]
        wqk_h = np.ascontiguousarray(
            np.concatenate([wq_h, wk_h], axis=1)).astype(bf)
        wkv_h = np.ascontiguousarray(
            np.concatenate([wk_h, wv_h], axis=1)).astype(bf)
        w2_h = W_out[hs, :]
        w2d_h = np.ascontiguousarray(
            np.concatenate([w2_h, w2_h], axis=0)).astype(bf)
        in_maps.append({
            "xT": xT, "wqk": wqk_h, "wkv": wkv_h, "w2d": w2d_h,
            "cosT": cosT, "sinT": sinT, "cosN": cosN, "sinN": sinN,
            "mask": mask, "ident": ident,
        })
    return in_maps, e_in


def combine_outputs(results, b_out):
    b_out = np.asarray(b_out, dtype=np.float32)
    acc = np.zeros((E, L), np.float32)
    for r in results:
        acc += np.asarray(r["outP"]).astype(np.float32)
    # unpermute column slots -> chunk order
    out = np.empty((E, L), np.float32)
    for s, ch in enumerate(CHUNK_OF_SLOT):
        out[:, ch * C : (ch + 1) * C] = acc[:, s * C : (s + 1) * C]
    out = out.T + b_out[None, :]
    return out.reshape(B, L, E).astype(np.float32)


_PROGRAM_CACHE = {}


def _get_program(e_in):
    if e_in not in _PROGRAM_CACHE:
        _PROGRAM_CACHE[e_in] = build_program(e_in=e_in)
    return _PROGRAM_CACHE[e_in]


def kernel(x, W_qkv, b_qkv, W_out, b_out):
    from concourse.bass_utils import run_bass_kernel_spmd

    in_maps, e_in = prepare_in_maps(x, W_qkv, b_qkv, W_out)
    nc = _get_program(e_in)
    res = run_bass_kernel_spmd(nc, in_maps, core_ids=list(range(N_CORES)))
    return combine_outputs(res.results, b_out)



# revision 5
# speedup vs baseline: 649.6366x; 649.6366x over previous
"""Cosformer causal attention (B=1, L=2048, E=512, H=8) on 8 TRN2 NeuronCores.

One head per core; single raw-x projection streams; trig folded into
constants wherever the cosformer algebra allows:
  A[l, m] = relu(q_l).relu(k_m) * cos(th_l - th_m)   (th = pi/2 * pos/L)

Per core (head h), from raw xT [e_in, L] (bf16):
  T-stream : psum = [Wq|Wk]^T xT -> relu -> qk [128, L] (q rows 0-63, k 64-127)
  scalings : qcqs = [q'*cos ; q'*sin] via 2 DVE ops + DMA partition shift;
             kraw = k' shifted to partitions 0-63 (DMA); no scaled k needed --
             the intra-chunk cos(th_l - th_m) is TOEPLITZ, folded into the
             causal mask constant (same [C, C] matrix for every chunk).
  N-stream : per chunk psum = xT_chunk^T [Wk|Wv] -> krelu, v natural;
             kn = [k'*cos | k'*sin] (free-dim packed, gpsimd), vaug = [v, 1]
  P2 (C=128 chunks): s0 = kraw^T q' ; at = s0 * maskcos (DVE);
             O = at^T vaug + qcqs^T kvsnap (K=128 packs cos+sin branches);
             KV state accumulates in a persistent PSUM bank (kn^T vaug, one
             M=128-packed matmul/chunk) with per-chunk bf16 snapshots; the
             state chain is emitted ahead of each group so its latency
             overlaps the S0/intra work.
  P3: obf = O[:, :64]/(O[:, 64]+eps); PAIRED PE transposes (2 chunks ->
      [128, C], even chunk rows 0-63 / odd rows 64-127) -> otb [128, 8, C];
      two-row-stream out-projection against w2dup (W_out head rows duplicated
      to both partition halves); outP columns are chunk-permuted
      (CHUNK_OF_SLOT; host unpermutes).
Emission is software-pipelined: body i+1's DMAs/projections are interleaved
into body i's P2 groups (in-order engines fill dependency stalls).
Host: out = sum_h unperm(outP_h)^T + b_out.  rel err ~4.6e-3 vs reference.
"""

import numpy as np
import ml_dtypes

import concourse.bass as bass
import concourse.mybir as mybir
from concourse.tile import TileContext
from concourse.vector_clock import ScopedClock

BF16 = mybir.dt.bfloat16
F32 = mybir.dt.float32
AF = mybir.ActivationFunctionType
ALU = mybir.AluOpType

B, L, E, H = 1, 2048, 512, 8
D = E // H            # 64 head dim
C = 128               # chunk length
NCH = L // C          # 16 chunks
GRP = 4               # chunks per psum group
NG = NCH // GRP       # 4 groups
LT = 512              # l tile for N=512 matmul streams
NLT = L // LT         # 4
EPS = 1e-6
N_CORES = 8

# outP column slot s holds chunk CHUNK_OF_SLOT[s] (see P3 pairing)
CHUNK_OF_SLOT = [0, 2, 4, 6, 1, 3, 5, 7, 8, 10, 12, 14, 9, 11, 13, 15]


def _split_multi_waits(bir_json):
    """The walrus in this container accepts at most ONE sem wait per
    instruction; split extras into standalone EventSemaphore waits placed
    immediately before the instruction (same engine => order preserved)."""
    import json as _json

    js = _json.loads(bir_json)
    ctr = 0
    for fn in js.get("functions", []):
        for bb in fn.get("blocks", []):
            insts = bb.get("instructions")
            if not insts:
                continue
            out = []
            changed = False
            for inst in insts:
                si = inst.get("sync_info")
                waits = si.get("on_wait", []) if si else []
                if len(waits) > 1:
                    changed = True
                    for w in waits[:-1]:
                        ctr += 1
                        out.append({
                            "debug": inst.get("debug", 0),
                            "engine": inst["engine"],
                            "ins": [],
                            "name": f"I-splitw-{ctr}",
                            "opcode": "EventSemaphore",
                            "outs": [],
                            "sync_info": {"on_update": [], "on_wait": [w]},
                        })
                    si["on_wait"] = [waits[-1]]
                out.append(inst)
            if changed:
                bb["instructions"] = out
    return _json.dumps(js).encode()


def _install_wait_split_hook():
    import concourse.bass2jax as bass2jax
    import concourse.bass_utils as bass_utils

    if getattr(bass2jax, "_wait_split_installed", False):
        return
    orig = bass_utils.compile_bir_kernel

    def patched(bir_json, tmpdir, neff_name="file.neff"):
        return orig(_split_multi_waits(bir_json), tmpdir, neff_name=neff_name)

    bass2jax.compile_bir_kernel = patched
    bass_utils.compile_bir_kernel = patched
    bass2jax._wait_split_installed = True


_install_wait_split_hook()


class SplitDrainTileContext(TileContext):
    """walrus in this container rejects >1 sem wait on the final SP Drain;
    spread the accumulated waits over single-wait SP wait instructions."""

    def _drain_and_barrier(self, tick_clock, wait_clock):
        nc = self.nc
        drain_inst = nc.sync.drain()
        wait_clock.add_sem_waits(
            drain_inst.ins, ScopedClock({None: tick_clock.global_clock})
        )
        si = drain_inst.ins.sync_info
        waits = list(si.on_wait) if si is not None else []
        if len(waits) > 1:
            drain_inst.ins.sync_info.on_wait = waits[:1]
            name2sem = {v.name: v for v in self.sems.allocated().values()}
            for w in waits[1:]:
                nc.sync.wait_ge(name2sem[w.ant_name], w.wait_value)
        nc.all_engine_barrier()
        popped = nc._tile_sem_poison_stack.pop()
        assert popped is self._sem_poison
        nc.clear_and_free_semaphores(list(self.sems.allocated().values()))
        nc.all_engine_barrier()


def build_program(e_in=E, repeat=1, debug=False, hw_loop=False):
    """hw_loop=True runs `repeat` bodies via a For_i hardware loop (constant
    program size); hw_loop=False unrolls them (legacy)."""
    nc = bass.Bass("TRN2", target_bir_lowering=False, debug=False,
                   num_devices=N_CORES)

    ecs = [(i, 128) for i in range(4)]
    if e_in > 4 * 128:
        assert e_in == 4 * 128 + 1
        ecs.append((4, e_in - 4 * 128))
    NEC = len(ecs)

    xT = nc.dram_tensor("xT", [e_in, L], BF16, kind="ExternalInput")
    wqk = nc.dram_tensor("wqk", [e_in, 128], BF16, kind="ExternalInput")
    wkv = nc.dram_tensor("wkv", [e_in, 128], BF16, kind="ExternalInput")
    w2d = nc.dram_tensor("w2d", [128, E], BF16, kind="ExternalInput")
    cosTd = nc.dram_tensor("cosT", [1, L], BF16, kind="ExternalInput")
    sinTd = nc.dram_tensor("sinT", [1, L], BF16, kind="ExternalInput")
    cosNd = nc.dram_tensor("cosN", [C, NCH], BF16, kind="ExternalInput")
    sinNd = nc.dram_tensor("sinN", [C, NCH], BF16, kind="ExternalInput")
    maskd = nc.dram_tensor("mask", [C, C], BF16, kind="ExternalInput")
    identd = nc.dram_tensor("ident", [C, C], BF16, kind="ExternalInput")
    outP = nc.dram_tensor("outP", [E, L], BF16, kind="ExternalOutput")

    with SplitDrainTileContext(nc) as tc:
        with (
            tc.tile_pool(name="const", bufs=1) as cpool,
            tc.tile_pool(name="work", bufs=2) as wpool,
            tc.tile_pool(name="stage", bufs=3) as spool,
            tc.tile_pool(name="pbig", bufs=2, space="PSUM") as pbig,
            tc.tile_pool(name="pso", bufs=2, space="PSUM") as pso,
            tc.tile_pool(name="pkv", bufs=2, space="PSUM") as pkv,
        ):
            # ---- constants (one slot, DMA'd once) ----
            wqk_sb = cpool.tile([128, NEC, 128], BF16, tag="wqk")
            wkv_sb = cpool.tile([128, NEC, 128], BF16, tag="wkv")
            w2d_sb = cpool.tile([128, NLT, 128], BF16, tag="w2d")
            cosT_sb = cpool.tile([128, L], BF16, tag="cosT")
            sinT_sb = cpool.tile([128, L], BF16, tag="sinT")
            cosN_sb = cpool.tile([C, NCH], BF16, tag="cosN")
            sinN_sb = cpool.tile([C, NCH], BF16, tag="sinN")
            mask_sb = cpool.tile([C, C], BF16, tag="mask")
            ident_sb = cpool.tile([C, C], BF16, tag="ident")

            for t_sb, t_d in [(wqk_sb, wqk), (wkv_sb, wkv)]:
                nc.sync.dma_start(
                    t_sb[:, :4, :],
                    t_d[: 4 * 128, :].rearrange("(c p) d -> p c d", p=128),
                )
                if NEC == 5:
                    nc.sync.dma_start(t_sb[:1, 4, :], t_d[4 * 128 :, :])
            nc.sync.dma_start(
                cosT_sb[:], cosTd[:1, :].to_broadcast([128, L]))
            nc.sync.dma_start(
                sinT_sb[:], sinTd[:1, :].to_broadcast([128, L]))
            nc.sync.dma_start(
                w2d_sb[:], w2d.rearrange("d (t n) -> d t n", n=128)
            )
            nc.sync.dma_start(cosN_sb[:], cosNd[:])
            nc.sync.dma_start(sinN_sb[:], sinNd[:])
            nc.sync.dma_start(mask_sb[:], maskd[:])
            nc.sync.dma_start(ident_sb[:], identd[:])

            def make_tiles():
                tl = {}
                tl["xT"] = wpool.tile([128, NEC, L], BF16, tag="xT", name="xTs")
                tl["qk"] = wpool.tile([128, L], BF16, tag="qk", name="qkt")
                tl["qcqs"] = wpool.tile([128, L], BF16, tag="qcqs", name="qcqst")
                tl["kraw"] = wpool.tile([D, L], BF16, tag="kraw", name="krawt")
                tl["qstmp"] = wpool.tile([D, L], BF16, tag="qstmp", name="qstmpt")
                tl["krelu"] = wpool.tile([C, NCH, D], BF16, tag="krelu", name="krelut")
                tl["kn"] = wpool.tile([C, NCH, 128], BF16, tag="kn", name="knt")
                tl["vaug"] = wpool.tile([C, NCH, D + 1], BF16, tag="vaug", name="vaugt")
                tl["at"] = wpool.tile([C, NCH, C], BF16, tag="at", name="att")
                tl["kvs"] = wpool.tile([128, NCH - 1, D + 1], BF16, tag="kvs", name="kvst")
                tl["obf"] = wpool.tile([C, NCH, D], BF16, tag="obf", name="obft")
                tl["otb"] = wpool.tile([128, NCH // 2, C], BF16, tag="otb", name="otbt")
                tl["rtmp"] = wpool.tile([C, NCH], F32, tag="rtmp", name="rtmpt")
                tl["r"] = wpool.tile([C, NCH], F32, tag="r", name="rt")
                return tl

            def a_parts(tl):
                """Input DMA + projections + scalings for one body, as a list
                of emission thunks (interleaved into the previous body)."""
                parts = []

                def dma_x(t, eng):
                    def f():
                        ls = slice(t * LT, (t + 1) * LT)
                        eng.dma_start(
                            tl["xT"][:, :4, ls],
                            xT[: 4 * 128, ls].rearrange(
                                "(c p) l -> p c l", p=128),
                        )
                        if NEC == 5 and t == 0:
                            eng.dma_start(tl["xT"][:1, 4, :], xT[4 * 128 :, :])
                        if t == 0:
                            nc.gpsimd.memset(tl["vaug"][:, :, D : D + 1], 1.0)
                    return f

                def tproj(tp):
                    def f():
                        pT = pbig.tile([128, 2, LT], F32, tag="big",
                                       name="pT")
                        for half in range(2):
                            t = 2 * tp + half
                            ls = slice(t * LT, (t + 1) * LT)
                            for i, (ec, pc) in enumerate(ecs):
                                nc.tensor.matmul(
                                    pT[:, half, :], wqk_sb[:pc, ec, :],
                                    tl["xT"][:pc, ec, ls],
                                    start=(i == 0), stop=(i == NEC - 1),
                                )
                        ls2 = slice(tp * 2 * LT, (tp + 1) * 2 * LT)
                        if tp == 0:
                            nc.scalar.activation(tl["qk"][:, ls2], pT[:],
                                                 AF.Relu)
                        else:
                            nc.vector.tensor_scalar_max(tl["qk"][:, ls2],
                                                        pT[:], 0.0)
                    return f

                def scal(hl):
                    def f():
                        ls = slice(hl * (L // 2), (hl + 1) * (L // 2))
                        nc.vector.tensor_tensor(
                            tl["qcqs"][:D, ls], tl["qk"][:D, ls],
                            cosT_sb[:D, ls], ALU.mult,
                        )
                        nc.vector.tensor_tensor(
                            tl["qstmp"][:, ls], tl["qk"][:D, ls],
                            sinT_sb[:D, ls], ALU.mult,
                        )
                        for q in range(2):
                            qs_ = slice((2 * hl + q) * LT,
                                        (2 * hl + q + 1) * LT)
                            nc.sync.dma_start(tl["kraw"][:, qs_],
                                              tl["qk"][D:, qs_])
                            nc.scalar.dma_start(tl["qcqs"][D:, qs_],
                                                tl["qstmp"][:, qs_])
                    return f

                def nstream(h):
                    def f():
                        pN = pbig.tile([C, 2 * GRP, 128], F32, tag="big",
                                       name="pN")
                        for sub in range(2 * GRP):
                            j = h * 2 * GRP + sub
                            cs = slice(j * C, (j + 1) * C)
                            for i, (ec, pc) in enumerate(ecs):
                                nc.tensor.matmul(
                                    pN[:, sub, :], tl["xT"][:pc, ec, cs],
                                    wkv_sb[:pc, ec, :],
                                    start=(i == 0), stop=(i == NEC - 1),
                                )
                        gs = slice(h * 2 * GRP, (h + 1) * 2 * GRP)
                        nc.scalar.activation(
                            tl["krelu"][:, gs, :], pN[:, :, :D], AF.Relu)
                        nc.scalar.activation(
                            tl["vaug"][:, gs, :D], pN[:, :, D:], AF.Copy)
                        nc.gpsimd.tensor_tensor(
                            tl["kn"][:, gs, :D], tl["krelu"][:, gs, :],
                            cosN_sb[:, gs, None].to_broadcast(
                                [C, 2 * GRP, D]),
                            ALU.mult,
                        )
                        nc.gpsimd.tensor_tensor(
                            tl["kn"][:, gs, D:], tl["krelu"][:, gs, :],
                            sinN_sb[:, gs, None].to_broadcast(
                                [C, 2 * GRP, D]),
                            ALU.mult,
                        )
                    return f

                # HWDGE engines only (sync/scalar): a gpsimd (SWDGE) DMA in
                # a For_i body makes the loop reset emit INC_SWDGE_SEM,
                # which this container's walrus cannot encode.
                parts.append(dma_x(0, nc.sync))
                parts.append(dma_x(1, nc.scalar))
                parts.append(dma_x(2, nc.sync))
                parts.append(dma_x(3, nc.scalar))
                parts.append(tproj(0))
                parts.append(scal(0))
                parts.append(tproj(1))
                parts.append(scal(1))
                parts.append(nstream(0))
                parts.append(nstream(1))
                return parts

            def state_steps(tl, kv_ps, g):
                # kv accumulation + snapshots for chunks of group g (emitted
                # ahead of the group's O work; chain overlaps S0/intra phase)
                for sub in range(GRP):
                    j = g * GRP + sub
                    if j < NCH - 1:
                        nc.tensor.matmul(
                            kv_ps[:, : D + 1], tl["kn"][:, j, :],
                            tl["vaug"][:, j, :],
                            start=(j == 0), stop=(j == NCH - 2),
                            skip_group_check=True,
                        )
                        if j % 2 == 0:
                            nc.scalar.activation(
                                tl["kvs"][:, j, :], kv_ps[:, : D + 1],
                                AF.Copy)
                        else:
                            nc.vector.tensor_copy(
                                tl["kvs"][:, j, :], kv_ps[:, : D + 1])

            def b_group(tl, g):
                gs = slice(g * GRP, (g + 1) * GRP)
                s0 = pso.tile([C, GRP, C], F32, tag="so")
                for sub in range(GRP):
                    j = g * GRP + sub
                    cs = slice(j * C, (j + 1) * C)
                    nc.tensor.matmul(
                        s0[:, sub, :], tl["kraw"][:, cs], tl["qk"][:D, cs],
                        start=True, stop=True,
                    )
                nc.vector.tensor_tensor(
                    tl["at"][:, gs, :], s0[:],
                    mask_sb[:, None, :].to_broadcast([C, GRP, C]), ALU.mult,
                )
                o_ps = pso.tile([C, GRP, 128], F32, tag="so")
                for sub in range(GRP):
                    j = g * GRP + sub
                    cs = slice(j * C, (j + 1) * C)
                    nc.tensor.matmul(
                        o_ps[:, sub, : D + 1], tl["at"][:, j, :],
                        tl["vaug"][:, j, :],
                        start=True, stop=(j == 0),
                    )
                    if j > 0:
                        nc.tensor.matmul(
                            o_ps[:, sub, : D + 1], tl["qcqs"][:, cs],
                            tl["kvs"][:, j - 1, :],
                            start=False, stop=True,
                        )
                nc.vector.tensor_scalar_add(
                    tl["rtmp"][:, gs], o_ps[:, :, D], EPS)
                nc.vector.reciprocal(tl["r"][:, gs], tl["rtmp"][:, gs])
                nc.vector.tensor_tensor(
                    tl["obf"][:, gs, :], o_ps[:, :, :D],
                    tl["r"][:, gs, None].to_broadcast([C, GRP, D]), ALU.mult,
                )

            def c_tr(tl, h2):
                tp = pso.tile([128, NCH // 4, C], BF16, tag="so", name="tp")
                for tt in range(NCH // 4):
                    t = h2 * (NCH // 4) + tt
                    nc.tensor.transpose(
                        tp[:, tt, :], tl["obf"][:, 2 * t : 2 * t + 2, :],
                        ident_sb[:],
                    )
                ts = slice(h2 * 4, (h2 + 1) * 4)
                nc.vector.tensor_copy(tl["otb"][:, ts, :], tp[:])

            def c_p3(tl, h2, nss):
                ts = slice(h2 * 4, (h2 + 1) * 4)
                for ns in nss:
                    pab = pbig.tile([128, 2, LT], F32, tag="big", name="pab")
                    nc.tensor.matmul(
                        pab[:, 0, :], w2d_sb[:D, ns, :], tl["otb"][:D, ts, :],
                        start=True, stop=True,
                    )
                    nc.tensor.matmul(
                        pab[:, 1, :], w2d_sb[D:, ns, :], tl["otb"][D:, ts, :],
                        start=True, stop=True,
                    )
                    ob = spool.tile([128, 2, LT], BF16, tag="ob")
                    if (h2 * NLT + ns) % 4 == 1:
                        nc.vector.tensor_copy(ob[:], pab[:])
                    else:
                        nc.scalar.activation(ob[:], pab[:], AF.Copy)
                    nc.sync.dma_start(
                        outP[ns * 128 : (ns + 1) * 128,
                             h2 * 1024 : (h2 + 1) * 1024],
                        ob[:],
                    )

            def one_body(tl):
                """Emit one full body, no cross-body pipelining."""
                for p in a_parts(tl):
                    p()
                kv_ps = pkv.tile([128, D + 1], F32, tag="kv")
                for g in range(NG):
                    state_steps(tl, kv_ps, g)
                    b_group(tl, g)
                    if g == 1:
                        c_tr(tl, 0)
                        c_p3(tl, 0, [0, 1, 2, 3])
                c_tr(tl, 1)
                c_p3(tl, 1, [0, 1, 2, 3])

            if hw_loop:
                cur = make_tiles()
                with tc.For_i(0, repeat):
                    one_body(cur)
                return nc

            # ---- software-pipelined emission across bodies ----
            cur = make_tiles()
            carry = []
            for p in a_parts(cur):
                p()
            for rep in range(repeat):
                if rep + 1 < repeat:
                    nxt = make_tiles()
                    pending = a_parts(nxt)
                else:
                    nxt, pending = None, []
                pi = iter(pending)

                def drain(n):
                    for _ in range(n):
                        p = next(pi, None)
                        if p is not None:
                            p()

                kv_ps = pkv.tile([128, D + 1], F32, tag="kv")
                for g in range(NG):
                    state_steps(cur, kv_ps, g)
                    b_group(cur, g)
                    if g == 1:
                        c_tr(cur, 0)
                        c_p3(cur, 0, [0, 1, 2, 3])
                    drain(3 if g < 2 else 2)
                c_tr(cur, 1)
                c_p3(cur, 1, [0, 1, 2, 3])
                for p in pi:
                    p()
                cur = nxt

    return nc


def prepare_in_maps(x, W_qkv, b_qkv, W_out):
    """Host-side sharding/layout prep. Returns (in_maps, e_in)."""
    x = np.asarray(x, dtype=np.float32).reshape(L, E)
    W_qkv = np.asarray(W_qkv, dtype=np.float32)
    b_qkv = np.asarray(b_qkv, dtype=np.float32)
    W_out = np.asarray(W_out, dtype=np.float32)

    use_bias = bool(np.any(b_qkv))
    if use_bias:
        x_aug = np.concatenate([x, np.ones((L, 1), np.float32)], axis=1)
        W_aug = np.concatenate([W_qkv, b_qkv[None, :]], axis=0)
    else:
        x_aug, W_aug = x, W_qkv
    e_in = x_aug.shape[1]

    bf = ml_dtypes.bfloat16
    xT = np.ascontiguousarray(x_aug.T).astype(bf)

    pos = np.arange(L, dtype=np.float32)
    theta = (np.pi / 2) * pos / L
    cosw = np.cos(theta).astype(np.float32)
    sinw = np.sin(theta).astype(np.float32)
    cosT = np.ascontiguousarray(cosw[None, :]).astype(bf)
    sinT = np.ascontiguousarray(sinw[None, :]).astype(bf)
    cosN = np.ascontiguousarray(cosw.reshape(NCH, C).T).astype(bf)
    sinN = np.ascontiguousarray(sinw.reshape(NCH, C).T).astype(bf)
    # mask[m, l] = cos(theta_l - theta_m) for m <= l else 0  (Toeplitz intra)
    lm = np.arange(C, dtype=np.float32)
    dth = (np.pi / 2) * (lm[None, :] - lm[:, None]) / L
    mask = (np.triu(np.ones((C, C), np.float32)) * np.cos(dth)).astype(bf)
    ident = np.eye(C, dtype=np.float32).astype(bf)

    in_maps = []
    for h in range(N_CORES):
        hs = slice(h * D, (h + 1) * D)
        wq_h = W_aug[:, hs]
        wk_h = W_aug[:, E + h * D : E + (h + 1) * D]
        wv_h = W_aug[:, 2 * E + h * D : 2 * E + (h + 1) * D]
        wqk_h = np.ascontiguousarray(
            np.concatenate([wq_h, wk_h], axis=1)).astype(bf)
        wkv_h = np.ascontiguousarray(
            np.concatenate([wk_h, wv_h], axis=1)).astype(bf)
        w2_h = W_out[hs, :]
        w2d_h = np.ascontiguousarray(
            np.concatenate([w2_h, w2_h], axis=0)).astype(bf)
        in_maps.append({
            "xT": xT, "wqk": wqk_h, "wkv": wkv_h, "w2d": w2d_h,
            "cosT": cosT, "sinT": sinT, "cosN": cosN, "sinN": sinN,
            "mask": mask, "ident": ident,
        })
    return in_maps, e_in


def combine_outputs(results, b_out):
    b_out = np.asarray(b_out, dtype=np.float32)
    acc = np.zeros((E, L), np.float32)
    for r in results:
        acc += np.asarray(r["outP"]).astype(np.float32)
    # unpermute column slots -> chunk order
    out = np.empty((E, L), np.float32)
    for s, ch in enumerate(CHUNK_OF_SLOT):
        out[:, ch * C : (ch + 1) * C] = acc[:, s * C : (s + 1) * C]
    out = out.T + b_out[None, :]
    return out.reshape(B, L, E).astype(np.float32)


_PROGRAM_CACHE = {}


def _get_program(e_in):
    if e_in not in _PROGRAM_CACHE:
        _PROGRAM_CACHE[e_in] = build_program(e_in=e_in)
    return _PROGRAM_CACHE[e_in]


def kernel(x, W_qkv, b_qkv, W_out, b_out):
    from concourse.bass_utils import run_bass_kernel_spmd

    in_maps, e_in = prepare_in_maps(x, W_qkv, b_qkv, W_out)
    nc = _get_program(e_in)
    res = run_bass_kernel_spmd(nc, in_maps, core_ids=list(range(N_CORES)))
    return combine_outputs(res.results, b_out)



# revision 16
# speedup vs baseline: 839.0050x; 1.2915x over previous
"""Cosformer causal attention (B=1, L=2048, E=512, H=8) on 8 TRN2 NeuronCores.

One head per core; single raw-x projection streams; trig folded into
constants wherever the cosformer algebra allows:
  A[l, m] = relu(q_l).relu(k_m) * cos(th_l - th_m)   (th = pi/2 * pos/L)

Per core (head h), from raw xT [e_in, L] (bf16):
  T-stream : psum = [Wq|Wk]^T xT -> relu -> qk [128, L] (q rows 0-63, k 64-127)
  scalings : qcqs = [q'*cos ; q'*sin] via 2 DVE ops + DMA partition shift;
             kraw = k' shifted to partitions 0-63 (DMA); no scaled k needed --
             the intra-chunk cos(th_l - th_m) is TOEPLITZ, folded into the
             causal mask constant (same [C, C] matrix for every chunk).
  N-stream : per chunk psum = xT_chunk^T [Wk|Wv] -> krelu, v natural;
             kn = [k'*cos | k'*sin] (free-dim packed, gpsimd), vaug = [v, 1]
  P2 (C=128 chunks): s0 = kraw^T q' ; at = s0 * maskcos (DVE);
             O = at^T vaug + qcqs^T kvsnap (K=128 packs cos+sin branches);
             KV state accumulates in a persistent PSUM bank (kn^T vaug, one
             M=128-packed matmul/chunk) with per-chunk bf16 snapshots; the
             state chain is emitted ahead of each group so its latency
             overlaps the S0/intra work.
  P3: obf = O[:, :64]/(O[:, 64]+eps); PAIRED PE transposes (2 chunks ->
      [128, C], even chunk rows 0-63 / odd rows 64-127) -> otb [128, 8, C];
      two-row-stream out-projection against w2dup (W_out head rows duplicated
      to both partition halves); outP columns are chunk-permuted
      (CHUNK_OF_SLOT; host unpermutes).
Emission is software-pipelined: body i+1's DMAs/projections are interleaved
into body i's P2 groups (in-order engines fill dependency stalls).
Host: out = sum_h unperm(outP_h)^T + b_out.  rel err ~4.6e-3 vs reference.
"""

import numpy as np
import ml_dtypes

import concourse.bass as bass
import concourse.mybir as mybir
from concourse.tile import TileContext
from concourse.vector_clock import ScopedClock

BF16 = mybir.dt.bfloat16
F32 = mybir.dt.float32
AF = mybir.ActivationFunctionType
ALU = mybir.AluOpType

B, L, E, H = 1, 2048, 512, 8
D = E // H            # 64 head dim
C = 128               # chunk length
NCH = L // C          # 16 chunks
GRP = 4               # chunks per psum group
NG = NCH // GRP       # 4 groups
LT = 512              # l tile for N=512 matmul streams
NLT = L // LT         # 4
EPS = 1e-6
N_CORES = 8

# outP column slot s holds chunk CHUNK_OF_SLOT[s] (see P3 pairing)
CHUNK_OF_SLOT = [0, 2, 4, 6, 1, 3, 5, 7, 8, 10, 12, 14, 9, 11, 13, 15]


def _split_multi_waits(bir_json):
    """The walrus in this container accepts at most ONE sem wait per
    instruction; split extras into standalone EventSemaphore waits placed
    immediately before the instruction (same engine => order preserved)."""
    import json as _json

    js = _json.loads(bir_json)
    ctr = 0
    for fn in js.get("functions", []):
        for bb in fn.get("blocks", []):
            insts = bb.get("instructions")
            if not insts:
                continue
            out = []
            changed = False
            for inst in insts:
                si = inst.get("sync_info")
                waits = si.get("on_wait", []) if si else []
                if len(waits) > 1:
                    changed = True
                    for w in waits[:-1]:
                        ctr += 1
                        out.append({
                            "debug": inst.get("debug", 0),
                            "engine": inst["engine"],
                            "ins": [],
                            "name": f"I-splitw-{ctr}",
                            "opcode": "EventSemaphore",
                            "outs": [],
                            "sync_info": {"on_update": [], "on_wait": [w]},
                        })
                    si["on_wait"] = [waits[-1]]
                out.append(inst)
            if changed:
                bb["instructions"] = out
    return _json.dumps(js).encode()


def _install_wait_split_hook():
    import concourse.bass2jax as bass2jax
    import concourse.bass_utils as bass_utils

    if getattr(bass2jax, "_wait_split_installed", False):
        return
    orig = bass_utils.compile_bir_kernel

    def patched(bir_json, tmpdir, neff_name="file.neff"):
        return orig(_split_multi_waits(bir_json), tmpdir, neff_name=neff_name)

    bass2jax.compile_bir_kernel = patched
    bass_utils.compile_bir_kernel = patched
    bass2jax._wait_split_installed = True


_install_wait_split_hook()


class SplitDrainTileContext(TileContext):
    """walrus in this container rejects >1 sem wait on the final SP Drain;
    spread the accumulated waits over single-wait SP wait instructions."""

    def _drain_and_barrier(self, tick_clock, wait_clock):
        nc = self.nc
        drain_inst = nc.sync.drain()
        wait_clock.add_sem_waits(
            drain_inst.ins, ScopedClock({None: tick_clock.global_clock})
        )
        si = drain_inst.ins.sync_info
        waits = list(si.on_wait) if si is not None else []
        if len(waits) > 1:
            drain_inst.ins.sync_info.on_wait = waits[:1]
            name2sem = {v.name: v for v in self.sems.allocated().values()}
            for w in waits[1:]:
                nc.sync.wait_ge(name2sem[w.ant_name], w.wait_value)
        nc.all_engine_barrier()
        popped = nc._tile_sem_poison_stack.pop()
        assert popped is self._sem_poison
        nc.clear_and_free_semaphores(list(self.sems.allocated().values()))
        nc.all_engine_barrier()


def build_program(e_in=E, repeat=1, debug=False, hw_loop=False):
    """hw_loop=True runs `repeat` bodies via a For_i hardware loop (constant
    program size); hw_loop=False unrolls them (legacy)."""
    nc = bass.Bass("TRN2", target_bir_lowering=False, debug=False,
                   num_devices=N_CORES)

    ecs = [(i, 128) for i in range(4)]
    if e_in > 4 * 128:
        assert e_in == 4 * 128 + 1
        ecs.append((4, e_in - 4 * 128))
    NEC = len(ecs)

    # x transposed, partition-major: xT[p, c*L + l] = x_aug[l, c*128 + p]
    # (one contiguous 16KB read per partition for the body's single x DMA)
    xT = nc.dram_tensor("xT", [128, 4 * L], BF16, kind="ExternalInput")
    if e_in > 4 * 128:
        assert e_in == 4 * 128 + 1
        xTb = nc.dram_tensor("xTb", [1, L], BF16, kind="ExternalInput")
    wqk = nc.dram_tensor("wqk", [e_in, 128], BF16, kind="ExternalInput")
    wkv = nc.dram_tensor("wkv", [e_in, 128], BF16, kind="ExternalInput")
    w2d = nc.dram_tensor("w2d", [128, E], BF16, kind="ExternalInput")
    cosTd = nc.dram_tensor("cosT", [1, L], BF16, kind="ExternalInput")
    sinTd = nc.dram_tensor("sinT", [1, L], BF16, kind="ExternalInput")
    cosNd = nc.dram_tensor("cosN", [C, NCH], BF16, kind="ExternalInput")
    sinNd = nc.dram_tensor("sinN", [C, NCH], BF16, kind="ExternalInput")
    maskd = nc.dram_tensor("mask", [C, C], BF16, kind="ExternalInput")
    identd = nc.dram_tensor("ident", [C, C], BF16, kind="ExternalInput")
    # outP[p, h2, n, l'] = column-slot-permuted out[n*128 + p, h2*1024 + l']
    # (one contiguous 8KB write per partition per h2-half store DMA)
    outP = nc.dram_tensor("outP", [128, 2, 4, 1024], BF16,
                          kind="ExternalOutput")

    with SplitDrainTileContext(nc) as tc:
        with (
            tc.tile_pool(name="const", bufs=1) as cpool,
            tc.tile_pool(name="work", bufs=2) as wpool,
            tc.tile_pool(name="stage", bufs=3) as spool,
            tc.tile_pool(name="pbig", bufs=2, space="PSUM") as pbig,
            tc.tile_pool(name="pso", bufs=2, space="PSUM") as pso,
            tc.tile_pool(name="pkv", bufs=2, space="PSUM") as pkv,
        ):
            # ---- constants (one slot, DMA'd once) ----
            wqk_sb = cpool.tile([128, NEC, 128], BF16, tag="wqk")
            wkv_sb = cpool.tile([128, NEC, 128], BF16, tag="wkv")
            w2d_sb = cpool.tile([128, NLT, 128], BF16, tag="w2d")
            cosT_sb = cpool.tile([128, L], BF16, tag="cosT")
            sinT_sb = cpool.tile([128, L], BF16, tag="sinT")
            cosN_sb = cpool.tile([C, NCH], BF16, tag="cosN")
            sinN_sb = cpool.tile([C, NCH], BF16, tag="sinN")
            mask_sb = cpool.tile([C, C], BF16, tag="mask")
            ident_sb = cpool.tile([C, C], BF16, tag="ident")

            for t_sb, t_d in [(wqk_sb, wqk), (wkv_sb, wkv)]:
                nc.sync.dma_start(
                    t_sb[:, :4, :],
                    t_d[: 4 * 128, :].rearrange("(c p) d -> p c d", p=128),
                )
                if NEC == 5:
                    nc.sync.dma_start(t_sb[:1, 4, :], t_d[4 * 128 :, :])
            nc.sync.dma_start(
                cosT_sb[:], cosTd[:1, :].to_broadcast([128, L]))
            nc.sync.dma_start(
                sinT_sb[:], sinTd[:1, :].to_broadcast([128, L]))
            nc.sync.dma_start(
                w2d_sb[:], w2d.rearrange("d (t n) -> d t n", n=128)
            )
            nc.sync.dma_start(cosN_sb[:], cosNd[:])
            nc.sync.dma_start(sinN_sb[:], sinNd[:])
            nc.sync.dma_start(mask_sb[:], maskd[:])
            nc.sync.dma_start(ident_sb[:], identd[:])

            def make_tiles():
                tl = {}
                tl["xT"] = wpool.tile([128, NEC, L], BF16, tag="xT", name="xTs")
                tl["qk"] = wpool.tile([128, L], BF16, tag="qk", name="qkt")
                tl["qcqs"] = wpool.tile([128, L], BF16, tag="qcqs", name="qcqst")
                tl["kraw"] = wpool.tile([D, L], BF16, tag="kraw", name="krawt")
                tl["qstmp"] = wpool.tile([D, L], BF16, tag="qstmp", name="qstmpt")
                tl["krelu"] = wpool.tile([C, NCH, D], BF16, tag="krelu", name="krelut")
                tl["kn"] = wpool.tile([C, NCH, 128], BF16, tag="kn", name="knt")
                tl["vaug"] = wpool.tile([C, NCH, D + 1], BF16, tag="vaug", name="vaugt")
                tl["at"] = wpool.tile([C, NCH, C], BF16, tag="at", name="att")
                tl["kvs"] = wpool.tile([128, NCH - 1, D + 1], BF16, tag="kvs", name="kvst")
                tl["obf"] = wpool.tile([C, NCH, D], BF16, tag="obf", name="obft")
                tl["otb"] = wpool.tile([128, NCH // 2, C], BF16, tag="otb", name="otbt")
                tl["rtmp"] = wpool.tile([C, NCH], F32, tag="rtmp", name="rtmpt")
                tl["r"] = wpool.tile([C, NCH], F32, tag="r", name="rt")
                return tl

            def a_parts(tl):
                """Input DMA + projections + scalings for one body, as a list
                of emission thunks (interleaved into the previous body)."""
                parts = []

                def dma_x(eng):
                    def f():
                        eng.dma_start(
                            tl["xT"][:, :4, :],
                            xT[:, :].rearrange("p (c l) -> p c l", l=L),
                        )
                        if NEC == 5:
                            eng.dma_start(tl["xT"][:1, 4, :], xTb[:, :])
                        nc.gpsimd.memset(tl["vaug"][:, :, D : D + 1], 1.0)
                    return f

                def tproj(tp):
                    def f():
                        pT = pbig.tile([128, 2, LT], F32, tag="big",
                                       name="pT")
                        for half in range(2):
                            t = 2 * tp + half
                            ls = slice(t * LT, (t + 1) * LT)
                            for i, (ec, pc) in enumerate(ecs):
                                nc.tensor.matmul(
                                    pT[:, half, :], wqk_sb[:pc, ec, :],
                                    tl["xT"][:pc, ec, ls],
                                    start=(i == 0), stop=(i == NEC - 1),
                                )
                        ls2 = slice(tp * 2 * LT, (tp + 1) * 2 * LT)
                        if tp == 0:
                            nc.scalar.activation(tl["qk"][:, ls2], pT[:],
                                                 AF.Relu)
                        else:
                            nc.vector.tensor_scalar_max(tl["qk"][:, ls2],
                                                        pT[:], 0.0)
                    return f

                def scal(hl):
                    def f():
                        ls = slice(hl * (L // 2), (hl + 1) * (L // 2))
                        nc.vector.tensor_tensor(
                            tl["qcqs"][:D, ls], tl["qk"][:D, ls],
                            cosT_sb[:D, ls], ALU.mult,
                        )
                        nc.vector.tensor_tensor(
                            tl["qstmp"][:, ls], tl["qk"][:D, ls],
                            sinT_sb[:D, ls], ALU.mult,
                        )
                        nc.sync.dma_start(tl["kraw"][:, ls],
                                          tl["qk"][D:, ls])
                        nc.scalar.dma_start(tl["qcqs"][D:, ls],
                                            tl["qstmp"][:, ls])
                    return f

                def nstream(h):
                    def f():
                        pN = pbig.tile([C, 2 * GRP, 128], F32, tag="big",
                                       name="pN")
                        for sub in range(2 * GRP):
                            j = h * 2 * GRP + sub
                            cs = slice(j * C, (j + 1) * C)
                            for i, (ec, pc) in enumerate(ecs):
                                nc.tensor.matmul(
                                    pN[:, sub, :], tl["xT"][:pc, ec, cs],
                                    wkv_sb[:pc, ec, :],
                                    start=(i == 0), stop=(i == NEC - 1),
                                )
                        gs = slice(h * 2 * GRP, (h + 1) * 2 * GRP)
                        nc.scalar.activation(
                            tl["krelu"][:, gs, :], pN[:, :, :D], AF.Relu)
                        nc.scalar.activation(
                            tl["vaug"][:, gs, :D], pN[:, :, D:], AF.Copy)
                        nc.gpsimd.tensor_tensor(
                            tl["kn"][:, gs, :D], tl["krelu"][:, gs, :],
                            cosN_sb[:, gs, None].to_broadcast(
                                [C, 2 * GRP, D]),
                            ALU.mult,
                        )
                        nc.gpsimd.tensor_tensor(
                            tl["kn"][:, gs, D:], tl["krelu"][:, gs, :],
                            sinN_sb[:, gs, None].to_broadcast(
                                [C, 2 * GRP, D]),
                            ALU.mult,
                        )
                    return f

                # HWDGE engines only (sync/scalar): a gpsimd (SWDGE) DMA in
                # a For_i body makes the loop reset emit INC_SWDGE_SEM,
                # which this container's walrus cannot encode.
                parts.append(dma_x(nc.sync))
                parts.append(tproj(0))
                parts.append(scal(0))
                parts.append(tproj(1))
                parts.append(scal(1))
                parts.append(nstream(0))
                parts.append(nstream(1))
                return parts

            def state_steps(tl, kv_ps, g):
                # kv accumulation + snapshots for chunks of group g (emitted
                # ahead of the group's O work; chain overlaps S0/intra phase)
                for sub in range(GRP):
                    j = g * GRP + sub
                    if j < NCH - 1:
                        nc.tensor.matmul(
                            kv_ps[:, : D + 1], tl["kn"][:, j, :],
                            tl["vaug"][:, j, :],
                            start=(j == 0), stop=(j == NCH - 2),
                            skip_group_check=True,
                        )
                        if j % 2 == 0:
                            nc.scalar.activation(
                                tl["kvs"][:, j, :], kv_ps[:, : D + 1],
                                AF.Copy)
                        else:
                            nc.vector.tensor_copy(
                                tl["kvs"][:, j, :], kv_ps[:, : D + 1])

            def b_group(tl, g):
                gs = slice(g * GRP, (g + 1) * GRP)
                s0 = pso.tile([C, GRP, C], F32, tag="so")
                for sub in range(GRP):
                    j = g * GRP + sub
                    cs = slice(j * C, (j + 1) * C)
                    nc.tensor.matmul(
                        s0[:, sub, :], tl["kraw"][:, cs], tl["qk"][:D, cs],
                        start=True, stop=True,
                    )
                nc.vector.tensor_tensor(
                    tl["at"][:, gs, :], s0[:],
                    mask_sb[:, None, :].to_broadcast([C, GRP, C]), ALU.mult,
                )
                o_ps = pso.tile([C, GRP, 128], F32, tag="so")
                for sub in range(GRP):
                    j = g * GRP + sub
                    cs = slice(j * C, (j + 1) * C)
                    nc.tensor.matmul(
                        o_ps[:, sub, : D + 1], tl["at"][:, j, :],
                        tl["vaug"][:, j, :],
                        start=True, stop=(j == 0),
                    )
                    if j > 0:
                        nc.tensor.matmul(
                            o_ps[:, sub, : D + 1], tl["qcqs"][:, cs],
                            tl["kvs"][:, j - 1, :],
                            start=False, stop=True,
                        )
                nc.vector.tensor_scalar_add(
                    tl["rtmp"][:, gs], o_ps[:, :, D], EPS)
                nc.vector.reciprocal(tl["r"][:, gs], tl["rtmp"][:, gs])
                nc.vector.tensor_tensor(
                    tl["obf"][:, gs, :], o_ps[:, :, :D],
                    tl["r"][:, gs, None].to_broadcast([C, GRP, D]), ALU.mult,
                )

            def c_tr(tl, h2):
                tp = pso.tile([128, NCH // 4, C], BF16, tag="so", name="tp")
                for tt in range(NCH // 4):
                    t = h2 * (NCH // 4) + tt
                    nc.tensor.transpose(
                        tp[:, tt, :], tl["obf"][:, 2 * t : 2 * t + 2, :],
                        ident_sb[:],
                    )
                ts = slice(h2 * 4, (h2 + 1) * 4)
                nc.vector.tensor_copy(tl["otb"][:, ts, :], tp[:])

            def c_p3(tl, h2, nss):
                ts = slice(h2 * 4, (h2 + 1) * 4)
                ob = spool.tile([128, 4, 2, LT], BF16, tag="ob")
                for ns in nss:
                    pab = pbig.tile([128, 2, LT], F32, tag="big", name="pab")
                    nc.tensor.matmul(
                        pab[:, 0, :], w2d_sb[:D, ns, :], tl["otb"][:D, ts, :],
                        start=True, stop=True,
                    )
                    nc.tensor.matmul(
                        pab[:, 1, :], w2d_sb[D:, ns, :], tl["otb"][D:, ts, :],
                        start=True, stop=True,
                    )
                    if (h2 * NLT + ns) % 4 == 1:
                        nc.vector.tensor_copy(ob[:, ns], pab[:])
                    else:
                        nc.scalar.activation(ob[:, ns], pab[:], AF.Copy)
                nc.sync.dma_start(
                    outP[:, h2, :, :],
                    ob[:].rearrange("p n a b -> p n (a b)"),
                )

            def one_body(tl, pending=()):
                """Emit one body's compute, interleaving `pending` emission
                thunks (the next body's input DMA/projections) into the
                group structure so in-order engines fill dependency stalls."""
                pi = iter(pending)

                def drain(n):
                    for _ in range(n):
                        p = next(pi, None)
                        if p is not None:
                            p()

                kv_ps = pkv.tile([128, D + 1], F32, tag="kv")
                for g in range(NG):
                    state_steps(tl, kv_ps, g)
                    b_group(tl, g)
                    if g == 1:
                        c_tr(tl, 0)
                        c_p3(tl, 0, [0, 1, 2, 3])
                    drain(2)
                c_tr(tl, 1)
                c_p3(tl, 1, [0, 1, 2, 3])
                for p in pi:
                    p()

            if hw_loop:
                if repeat == 1:
                    cur = make_tiles()
                    for p in a_parts(cur):
                        p()
                    one_body(cur)
                    return nc
                assert repeat % 2 == 0, "hw_loop repeat must be 1 or even"
                tA = make_tiles()
                tB = make_tiles()
                for p in a_parts(tA):
                    p()
                with tc.For_i(0, repeat // 2):
                    one_body(tA, a_parts(tB))
                    one_body(tB, a_parts(tA))
                return nc

            # ---- software-pipelined emission across unrolled bodies ----
            cur = make_tiles()
            for p in a_parts(cur):
                p()
            for rep in range(repeat):
                if rep + 1 < repeat:
                    nxt = make_tiles()
                    pending = a_parts(nxt)
                else:
                    nxt, pending = None, []
                one_body(cur, pending)
                cur = nxt

    return nc


def prepare_in_maps(x, W_qkv, b_qkv, W_out):
    """Host-side sharding/layout prep. Returns (in_maps, e_in)."""
    x = np.asarray(x, dtype=np.float32).reshape(L, E)
    W_qkv = np.asarray(W_qkv, dtype=np.float32)
    b_qkv = np.asarray(b_qkv, dtype=np.float32)
    W_out = np.asarray(W_out, dtype=np.float32)

    use_bias = bool(np.any(b_qkv))
    if use_bias:
        x_aug = np.concatenate([x, np.ones((L, 1), np.float32)], axis=1)
        W_aug = np.concatenate([W_qkv, b_qkv[None, :]], axis=0)
    else:
        x_aug, W_aug = x, W_qkv
    e_in = x_aug.shape[1]

    bf = ml_dtypes.bfloat16
    # partition-major: xT[p, c*L + l] = x_aug[l, c*128 + p]
    xTf = np.ascontiguousarray(x_aug.T).astype(bf)          # [e_in, L]
    xT = np.ascontiguousarray(
        xTf[: 4 * 128].reshape(4, 128, L).transpose(1, 0, 2).reshape(
            128, 4 * L))
    xTb = np.ascontiguousarray(xTf[4 * 128 :]) if e_in > 4 * 128 else None

    pos = np.arange(L, dtype=np.float32)
    theta = (np.pi / 2) * pos / L
    cosw = np.cos(theta).astype(np.float32)
    sinw = np.sin(theta).astype(np.float32)
    cosT = np.ascontiguousarray(cosw[None, :]).astype(bf)
    sinT = np.ascontiguousarray(sinw[None, :]).astype(bf)
    cosN = np.ascontiguousarray(cosw.reshape(NCH, C).T).astype(bf)
    sinN = np.ascontiguousarray(sinw.reshape(NCH, C).T).astype(bf)
    # mask[m, l] = cos(theta_l - theta_m) for m <= l else 0  (Toeplitz intra)
    lm = np.arange(C, dtype=np.float32)
    dth = (np.pi / 2) * (lm[None, :] - lm[:, None]) / L
    mask = (np.triu(np.ones((C, C), np.float32)) * np.cos(dth)).astype(bf)
    ident = np.eye(C, dtype=np.float32).astype(bf)

    in_maps = []
    for h in range(N_CORES):
        hs = slice(h * D, (h + 1) * D)
        wq_h = W_aug[:, hs]
        wk_h = W_aug[:, E + h * D : E + (h + 1) * D]
        wv_h = W_aug[:, 2 * E + h * D : 2 * E + (h + 1) * D]
        wqk_h = np.ascontiguousarray(
            np.concatenate([wq_h, wk_h], axis=1)).astype(bf)
        wkv_h = np.ascontiguousarray(
            np.concatenate([wk_h, wv_h], axis=1)).astype(bf)
        w2_h = W_out[hs, :]
        w2d_h = np.ascontiguousarray(
            np.concatenate([w2_h, w2_h], axis=0)).astype(bf)
        im = {
            "xT": xT, "wqk": wqk_h, "wkv": wkv_h, "w2d": w2d_h,
            "cosT": cosT, "sinT": sinT, "cosN": cosN, "sinN": sinN,
            "mask": mask, "ident": ident,
        }
        if xTb is not None:
            im["xTb"] = xTb
        in_maps.append(im)
    return in_maps, e_in


def combine_outputs(results, b_out):
    b_out = np.asarray(b_out, dtype=np.float32)
    accP = np.zeros((128, 2, 4, 1024), np.float32)
    for r in results:
        accP += np.asarray(r["outP"]).astype(np.float32)
    # outP[p, h2, n, l'] -> acc[n*128+p, h2*1024+l']
    acc = np.ascontiguousarray(
        accP.transpose(2, 0, 1, 3).reshape(E, L))
    # unpermute column slots -> chunk order
    out = np.empty((E, L), np.float32)
    for s, ch in enumerate(CHUNK_OF_SLOT):
        out[:, ch * C : (ch + 1) * C] = acc[:, s * C : (s + 1) * C]
    out = out.T + b_out[None, :]
    return out.reshape(B, L, E).astype(np.float32)


_PROGRAM_CACHE = {}


def _get_program(e_in):
    if e_in not in _PROGRAM_CACHE:
        _PROGRAM_CACHE[e_in] = build_program(e_in=e_in)
    return _PROGRAM_CACHE[e_in]


def kernel(x, W_qkv, b_qkv, W_out, b_out):
    from concourse.bass_utils import run_bass_kernel_spmd

    in_maps, e_in = prepare_in_maps(x, W_qkv, b_qkv, W_out)
    nc = _get_program(e_in)
    res = run_bass_kernel_spmd(nc, in_maps, core_ids=list(range(N_CORES)))
    return combine_outputs(res.results, b_out)



# revision 21
# speedup vs baseline: 857.1666x; 1.0216x over previous
"""Cosformer causal attention (B=1, L=2048, E=512, H=8) on 8 TRN2 NeuronCores.

One head per core; single raw-x projection streams; trig folded into
constants wherever the cosformer algebra allows:
  A[l, m] = relu(q_l).relu(k_m) * cos(th_l - th_m)   (th = pi/2 * pos/L)

Per core (head h), from raw xT [e_in, L] (bf16):
  T-stream : psum = [Wq|Wk]^T xT -> relu -> qk [128, L] (q rows 0-63, k 64-127)
  scalings : qcqs = [q'*cos ; q'*sin] via 2 DVE ops + DMA partition shift;
             kraw = k' shifted to partitions 0-63 (DMA); no scaled k needed --
             the intra-chunk cos(th_l - th_m) is TOEPLITZ, folded into the
             causal mask constant (same [C, C] matrix for every chunk).
  N-stream : per chunk psum = xT_chunk^T [Wk|Wv] -> krelu, v natural;
             kn = [k'*cos | k'*sin] (free-dim packed, gpsimd), vaug = [v, 1]
  P2 (C=128 chunks): s0 = kraw^T q' ; at = s0 * maskcos (DVE);
             O = at^T vaug + qcqs^T kvsnap (K=128 packs cos+sin branches);
             KV state accumulates in a persistent PSUM bank (kn^T vaug, one
             M=128-packed matmul/chunk) with per-chunk bf16 snapshots; the
             state chain is emitted ahead of each group so its latency
             overlaps the S0/intra work.
  P3: obf = O[:, :64]/(O[:, 64]+eps); PAIRED PE transposes (2 chunks ->
      [128, C], even chunk rows 0-63 / odd rows 64-127) -> otb [128, 8, C];
      two-row-stream out-projection against w2dup (W_out head rows duplicated
      to both partition halves); outP columns are chunk-permuted
      (CHUNK_OF_SLOT; host unpermutes).
Emission is software-pipelined: body i+1's DMAs/projections are interleaved
into body i's P2 groups (in-order engines fill dependency stalls).
Host: out = sum_h unperm(outP_h)^T + b_out.  rel err ~4.6e-3 vs reference.
"""

import numpy as np
import ml_dtypes

import concourse.bass as bass
import concourse.mybir as mybir
from concourse.tile import TileContext
from concourse.vector_clock import ScopedClock

BF16 = mybir.dt.bfloat16
F32 = mybir.dt.float32
AF = mybir.ActivationFunctionType
ALU = mybir.AluOpType

B, L, E, H = 1, 2048, 512, 8
D = E // H            # 64 head dim
C = 128               # chunk length
NCH = L // C          # 16 chunks
GRP = 4               # chunks per psum group
NG = NCH // GRP       # 4 groups
LT = 512              # l tile for N=512 matmul streams
NLT = L // LT         # 4
EPS = 1e-6
N_CORES = 8

# outP column slot s holds chunk CHUNK_OF_SLOT[s] (see P3 pairing)
CHUNK_OF_SLOT = [0, 2, 4, 6, 1, 3, 5, 7, 8, 10, 12, 14, 9, 11, 13, 15]


def _split_multi_waits(bir_json):
    """The walrus in this container accepts at most ONE sem wait per
    instruction; split extras into standalone EventSemaphore waits placed
    immediately before the instruction (same engine => order preserved)."""
    import json as _json

    js = _json.loads(bir_json)
    ctr = 0
    for fn in js.get("functions", []):
        for bb in fn.get("blocks", []):
            insts = bb.get("instructions")
            if not insts:
                continue
            out = []
            changed = False
            for inst in insts:
                si = inst.get("sync_info")
                waits = si.get("on_wait", []) if si else []
                if len(waits) > 1:
                    changed = True
                    for w in waits[:-1]:
                        ctr += 1
                        out.append({
                            "debug": inst.get("debug", 0),
                            "engine": inst["engine"],
                            "ins": [],
                            "name": f"I-splitw-{ctr}",
                            "opcode": "EventSemaphore",
                            "outs": [],
                            "sync_info": {"on_update": [], "on_wait": [w]},
                        })
                    si["on_wait"] = [waits[-1]]
                out.append(inst)
            if changed:
                bb["instructions"] = out
    return _json.dumps(js).encode()


def _install_wait_split_hook():
    import concourse.bass2jax as bass2jax
    import concourse.bass_utils as bass_utils

    if getattr(bass2jax, "_wait_split_installed", False):
        return
    orig = bass_utils.compile_bir_kernel

    def patched(bir_json, tmpdir, neff_name="file.neff"):
        return orig(_split_multi_waits(bir_json), tmpdir, neff_name=neff_name)

    bass2jax.compile_bir_kernel = patched
    bass_utils.compile_bir_kernel = patched
    bass2jax._wait_split_installed = True


_install_wait_split_hook()


class SplitDrainTileContext(TileContext):
    """walrus in this container rejects >1 sem wait on the final SP Drain;
    spread the accumulated waits over single-wait SP wait instructions."""

    def _drain_and_barrier(self, tick_clock, wait_clock):
        nc = self.nc
        drain_inst = nc.sync.drain()
        wait_clock.add_sem_waits(
            drain_inst.ins, ScopedClock({None: tick_clock.global_clock})
        )
        si = drain_inst.ins.sync_info
        waits = list(si.on_wait) if si is not None else []
        if len(waits) > 1:
            drain_inst.ins.sync_info.on_wait = waits[:1]
            name2sem = {v.name: v for v in self.sems.allocated().values()}
            for w in waits[1:]:
                nc.sync.wait_ge(name2sem[w.ant_name], w.wait_value)
        nc.all_engine_barrier()
        popped = nc._tile_sem_poison_stack.pop()
        assert popped is self._sem_poison
        nc.clear_and_free_semaphores(list(self.sems.allocated().values()))
        nc.all_engine_barrier()


def build_program(e_in=E, repeat=1, debug=False, hw_loop=False, ablate=()):
    """hw_loop=True runs `repeat` bodies via a For_i hardware loop (constant
    program size); hw_loop=False unrolls them (legacy). `ablate` names body
    components to omit — timing-probe builds only, output is garbage."""
    abl = set(ablate)
    nc = bass.Bass("TRN2", target_bir_lowering=False, debug=False,
                   num_devices=N_CORES)

    ecs = [(i, 128) for i in range(4)]
    if e_in > 4 * 128:
        assert e_in == 4 * 128 + 1
        ecs.append((4, e_in - 4 * 128))
    NEC = len(ecs)

    # x transposed, partition-major: xT[p, c*L + l] = x_aug[l, c*128 + p]
    # (one contiguous 16KB read per partition for the body's single x DMA)
    xT = nc.dram_tensor("xT", [128, 4 * L], BF16, kind="ExternalInput")
    if e_in > 4 * 128:
        assert e_in == 4 * 128 + 1
        xTb = nc.dram_tensor("xTb", [1, L], BF16, kind="ExternalInput")
    wqk = nc.dram_tensor("wqk", [e_in, 128], BF16, kind="ExternalInput")
    wkv = nc.dram_tensor("wkv", [e_in, 128], BF16, kind="ExternalInput")
    w2d = nc.dram_tensor("w2d", [128, E], BF16, kind="ExternalInput")
    cosTd = nc.dram_tensor("cosT", [1, L], BF16, kind="ExternalInput")
    sinTd = nc.dram_tensor("sinT", [1, L], BF16, kind="ExternalInput")
    cosNd = nc.dram_tensor("cosN", [C, NCH], BF16, kind="ExternalInput")
    sinNd = nc.dram_tensor("sinN", [C, NCH], BF16, kind="ExternalInput")
    maskd = nc.dram_tensor("mask", [C, C], BF16, kind="ExternalInput")
    identd = nc.dram_tensor("ident", [C, C], BF16, kind="ExternalInput")
    # outP[p, h2, n, l'] = column-slot-permuted out[n*128 + p, h2*1024 + l']
    # (one contiguous 8KB write per partition per h2-half store DMA)
    outP = nc.dram_tensor("outP", [128, 2, 4, 1024], BF16,
                          kind="ExternalOutput")

    with SplitDrainTileContext(nc) as tc:
        with (
            tc.tile_pool(name="const", bufs=1) as cpool,
            tc.tile_pool(name="work", bufs=2) as wpool,
            tc.tile_pool(name="stage", bufs=3) as spool,
            tc.tile_pool(name="pbig", bufs=2, space="PSUM") as pbig,
            tc.tile_pool(name="pso", bufs=2, space="PSUM") as pso,
            tc.tile_pool(name="pkv", bufs=2, space="PSUM") as pkv,
        ):
            # ---- constants (one slot, DMA'd once) ----
            wqk_sb = cpool.tile([128, NEC, 128], BF16, tag="wqk")
            wkv_sb = cpool.tile([128, NEC, 128], BF16, tag="wkv")
            w2d_sb = cpool.tile([128, NLT, 128], BF16, tag="w2d")
            cosT_sb = cpool.tile([128, L], BF16, tag="cosT")
            sinT_sb = cpool.tile([128, L], BF16, tag="sinT")
            cosN_sb = cpool.tile([C, NCH], BF16, tag="cosN")
            sinN_sb = cpool.tile([C, NCH], BF16, tag="sinN")
            mask_sb = cpool.tile([C, C], BF16, tag="mask")
            ident_sb = cpool.tile([C, C], BF16, tag="ident")

            for t_sb, t_d in [(wqk_sb, wqk), (wkv_sb, wkv)]:
                nc.sync.dma_start(
                    t_sb[:, :4, :],
                    t_d[: 4 * 128, :].rearrange("(c p) d -> p c d", p=128),
                )
                if NEC == 5:
                    nc.sync.dma_start(t_sb[:1, 4, :], t_d[4 * 128 :, :])
            nc.sync.dma_start(
                cosT_sb[:], cosTd[:1, :].to_broadcast([128, L]))
            nc.sync.dma_start(
                sinT_sb[:], sinTd[:1, :].to_broadcast([128, L]))
            nc.sync.dma_start(
                w2d_sb[:], w2d.rearrange("d (t n) -> d t n", n=128)
            )
            nc.sync.dma_start(cosN_sb[:], cosNd[:])
            nc.sync.dma_start(sinN_sb[:], sinNd[:])
            nc.sync.dma_start(mask_sb[:], maskd[:])
            nc.sync.dma_start(ident_sb[:], identd[:])

            def make_tiles():
                tl = {}
                tl["xT"] = wpool.tile([128, NEC, L], BF16, tag="xT", name="xTs")
                tl["qk"] = wpool.tile([128, L], BF16, tag="qk", name="qkt")
                tl["qcqs"] = wpool.tile([128, L], BF16, tag="qcqs", name="qcqst")
                tl["qstmp"] = wpool.tile([D, L], BF16, tag="qstmp", name="qstmpt")
                tl["kvt"] = wpool.tile([128, L], BF16, tag="kvt", name="kvtt")
                tl["kn"] = wpool.tile([C, NCH, 128], BF16, tag="kn", name="knt")
                tl["ktv"] = wpool.tile([C, NCH, 129], BF16, tag="ktv", name="ktvt")
                tl["at"] = wpool.tile([C, NCH, C], BF16, tag="at", name="att")
                tl["kvs"] = wpool.tile([128, NCH - 1, D + 1], BF16, tag="kvs", name="kvst")
                tl["obf"] = wpool.tile([C, NCH, D], BF16, tag="obf", name="obft")
                tl["otb"] = wpool.tile([128, NCH // 2, C], BF16, tag="otb", name="otbt")
                tl["rtmp"] = wpool.tile([C, NCH], F32, tag="rtmp", name="rtmpt")
                tl["r"] = wpool.tile([C, NCH], F32, tag="r", name="rt")
                return tl

            def a_parts(tl):
                """Input DMA + projections + scalings for one body, as a list
                of emission thunks (interleaved into the previous body)."""
                parts = []

                def dma_x(eng):
                    def f():
                        if "xdma" in abl:
                            nc.gpsimd.memset(
                                tl["ktv"][:, :, 128:129], 1.0)
                            return
                        eng.dma_start(
                            tl["xT"][:, :4, :],
                            xT[:, :].rearrange("p (c l) -> p c l", l=L),
                        )
                        if NEC == 5:
                            eng.dma_start(tl["xT"][:1, 4, :], xTb[:, :])
                        nc.gpsimd.memset(tl["ktv"][:, :, 128:129], 1.0)
                    return f

                def tproj(tp):
                    def f():
                        if "tstream" in abl:
                            return
                        pT = pbig.tile([128, 2, LT], F32, tag="big",
                                       name="pT")
                        for half in range(2):
                            t = 2 * tp + half
                            ls = slice(t * LT, (t + 1) * LT)
                            for i, (ec, pc) in enumerate(ecs):
                                nc.tensor.matmul(
                                    pT[:, half, :], wqk_sb[:pc, ec, :],
                                    tl["xT"][:pc, ec, ls],
                                    start=(i == 0), stop=(i == NEC - 1),
                                )
                        ls2 = slice(tp * 2 * LT, (tp + 1) * 2 * LT)
                        if tp == 0:
                            nc.scalar.activation(tl["qk"][:, ls2], pT[:],
                                                 AF.Relu)
                        else:
                            nc.vector.tensor_scalar_max(tl["qk"][:, ls2],
                                                        pT[:], 0.0)
                    return f

                def scal(hl):
                    def f():
                        if "scal" in abl:
                            return
                        ls = slice(hl * (L // 2), (hl + 1) * (L // 2))
                        nc.vector.tensor_tensor(
                            tl["qcqs"][:D, ls], tl["qk"][:D, ls],
                            cosT_sb[:D, ls], ALU.mult,
                        )
                        nc.vector.tensor_tensor(
                            tl["qstmp"][:, ls], tl["qk"][:D, ls],
                            sinT_sb[:D, ls], ALU.mult,
                        )
                        if "shift" not in abl:
                            nc.scalar.dma_start(tl["qcqs"][D:, ls],
                                                tl["qstmp"][:, ls])
                    return f

                def kvtp(tp):
                    # second T-stream: kvt = [Wk|Wv]^T xT; k rows relu'd
                    def f():
                        if "nstream" in abl:
                            return
                        pT = pbig.tile([128, 2, LT], F32, tag="big",
                                       name="pKV")
                        for half in range(2):
                            t = 2 * tp + half
                            ls = slice(t * LT, (t + 1) * LT)
                            for i, (ec, pc) in enumerate(ecs):
                                nc.tensor.matmul(
                                    pT[:, half, :], wkv_sb[:pc, ec, :],
                                    tl["xT"][:pc, ec, ls],
                                    start=(i == 0), stop=(i == NEC - 1),
                                )
                        ls2 = slice(tp * 2 * LT, (tp + 1) * 2 * LT)
                        nc.scalar.activation(tl["kvt"][:D, ls2],
                                             pT[:D, :, :], AF.Relu)
                        nc.vector.tensor_copy(tl["kvt"][D:, ls2],
                                              pT[D:, :, :])
                    return f

                def ktr(h):
                    # paired transposes: chunk-row layout ktv[l, j, [k'|v]]
                    def f():
                        if "nstream" in abl:
                            return
                        gs = slice(h * 2 * GRP, (h + 1) * 2 * GRP)
                        ptr = pso.tile([C, 2 * GRP, 128], BF16, tag="so",
                                       name="ptr")
                        for sub in range(2 * GRP):
                            j = h * 2 * GRP + sub
                            cs = slice(j * C, (j + 1) * C)
                            nc.tensor.transpose(
                                ptr[:, sub, :], tl["kvt"][:, cs],
                                ident_sb[:],
                            )
                        if h == 0:
                            nc.scalar.activation(tl["ktv"][:, gs, :128],
                                                 ptr[:], AF.Copy)
                        else:
                            nc.vector.tensor_copy(tl["ktv"][:, gs, :128],
                                                  ptr[:])
                        nc.gpsimd.tensor_tensor(
                            tl["kn"][:, gs, :D], tl["ktv"][:, gs, :D],
                            cosN_sb[:, gs, None].to_broadcast(
                                [C, 2 * GRP, D]),
                            ALU.mult,
                        )
                        nc.gpsimd.tensor_tensor(
                            tl["kn"][:, gs, D:], tl["ktv"][:, gs, :D],
                            sinN_sb[:, gs, None].to_broadcast(
                                [C, 2 * GRP, D]),
                            ALU.mult,
                        )
                    return f

                # HWDGE engines only (sync/scalar): a gpsimd (SWDGE) DMA in
                # a For_i body makes the loop reset emit INC_SWDGE_SEM,
                # which this container's walrus cannot encode.
                parts.append(dma_x(nc.sync))
                parts.append(tproj(0))
                parts.append(scal(0))
                parts.append(tproj(1))
                parts.append(scal(1))
                parts.append(kvtp(0))
                parts.append(ktr(0))
                parts.append(kvtp(1))
                parts.append(ktr(1))
                return parts

            def state_steps(tl, kv_ps, g):
                # kv accumulation + snapshots for chunks of group g (emitted
                # ahead of the group's O work; chain overlaps S0/intra phase)
                if "state" in abl:
                    return
                for sub in range(GRP):
                    j = g * GRP + sub
                    if j < NCH - 1:
                        nc.tensor.matmul(
                            kv_ps[:, : D + 1], tl["kn"][:, j, :],
                            tl["ktv"][:, j, D:],
                            start=(j == 0), stop=(j == NCH - 2),
                            skip_group_check=True,
                        )
                        if j % 2 == 0:
                            nc.scalar.activation(
                                tl["kvs"][:, j, :], kv_ps[:, : D + 1],
                                AF.Copy)
                        else:
                            nc.vector.tensor_copy(
                                tl["kvs"][:, j, :], kv_ps[:, : D + 1])

            def b_group(tl, g):
                gs = slice(g * GRP, (g + 1) * GRP)
                if "s0" not in abl:
                    b_group_s0(tl, g, gs)
                if "ov" not in abl:
                    b_group_ov(tl, g, gs)

            def b_group_s0(tl, g, gs):
                s0 = pso.tile([C, GRP, C], F32, tag="so")
                for sub in range(GRP):
                    j = g * GRP + sub
                    cs = slice(j * C, (j + 1) * C)
                    nc.tensor.matmul(
                        s0[:, sub, :], tl["kvt"][:D, cs], tl["qk"][:D, cs],
                        start=True, stop=True,
                    )
                nc.vector.tensor_tensor(
                    tl["at"][:, gs, :], s0[:],
                    mask_sb[:, None, :].to_broadcast([C, GRP, C]), ALU.mult,
                )

            def b_group_ov(tl, g, gs):
                o_ps = pso.tile([C, GRP, 128], F32, tag="so")
                for sub in range(GRP):
                    j = g * GRP + sub
                    cs = slice(j * C, (j + 1) * C)
                    nc.tensor.matmul(
                        o_ps[:, sub, : D + 1], tl["at"][:, j, :],
                        tl["ktv"][:, j, D:],
                        start=True, stop=(j == 0),
                    )
                    if j > 0:
                        nc.tensor.matmul(
                            o_ps[:, sub, : D + 1], tl["qcqs"][:, cs],
                            tl["kvs"][:, j - 1, :],
                            start=False, stop=True,
                        )
                if "norm" in abl:
                    nc.scalar.activation(tl["obf"][:, gs, :],
                                         o_ps[:, :, :D], AF.Copy)
                    return
                nc.vector.tensor_scalar_add(
                    tl["rtmp"][:, gs], o_ps[:, :, D], EPS)
                nc.vector.reciprocal(tl["r"][:, gs], tl["rtmp"][:, gs])
                nc.vector.tensor_tensor(
                    tl["obf"][:, gs, :], o_ps[:, :, :D],
                    tl["r"][:, gs, None].to_broadcast([C, GRP, D]), ALU.mult,
                )

            def c_tr(tl, h2):
                if "ctr" in abl:
                    return
                tp = pso.tile([128, NCH // 4, C], BF16, tag="so", name="tp")
                for tt in range(NCH // 4):
                    t = h2 * (NCH // 4) + tt
                    nc.tensor.transpose(
                        tp[:, tt, :], tl["obf"][:, 2 * t : 2 * t + 2, :],
                        ident_sb[:],
                    )
                ts = slice(h2 * 4, (h2 + 1) * 4)
                nc.vector.tensor_copy(tl["otb"][:, ts, :], tp[:])

            def c_p3(tl, h2, nss):
                if "oproj" in abl:
                    return
                ts = slice(h2 * 4, (h2 + 1) * 4)
                ob = spool.tile([128, 4, 2, LT], BF16, tag="ob")
                for ns in nss:
                    pab = pbig.tile([128, 2, LT], F32, tag="big", name="pab")
                    nc.tensor.matmul(
                        pab[:, 0, :], w2d_sb[:D, ns, :], tl["otb"][:D, ts, :],
                        start=True, stop=True,
                    )
                    nc.tensor.matmul(
                        pab[:, 1, :], w2d_sb[D:, ns, :], tl["otb"][D:, ts, :],
                        start=True, stop=True,
                    )
                    if (h2 * NLT + ns) % 4 == 1:
                        nc.vector.tensor_copy(ob[:, ns], pab[:])
                    else:
                        nc.scalar.activation(ob[:, ns], pab[:], AF.Copy)
                if "store" not in abl:
                    nc.scalar.dma_start(
                        outP[:, h2, :, :],
                        ob[:].rearrange("p n a b -> p n (a b)"),
                    )

            def one_body(tl, pending=()):
                """Emit one body's compute, interleaving `pending` emission
                thunks (the next body's input DMA/projections) into the
                group structure so in-order engines fill dependency stalls."""
                pi = iter(pending)

                def drain(n):
                    for _ in range(n):
                        p = next(pi, None)
                        if p is not None:
                            p()

                kv_ps = pkv.tile([128, D + 1], F32, tag="kv")
                for g in range(NG):
                    state_steps(tl, kv_ps, g)
                    b_group(tl, g)
                    if g == 1:
                        c_tr(tl, 0)
                        c_p3(tl, 0, [0, 1, 2, 3])
                    drain(2)
                c_tr(tl, 1)
                c_p3(tl, 1, [0, 1, 2, 3])
                for p in pi:
                    p()

            if hw_loop:
                if repeat == 1:
                    cur = make_tiles()
                    for p in a_parts(cur):
                        p()
                    one_body(cur)
                    return nc
                assert repeat % 4 == 0, (
                    "hw_loop repeat must be 1 or a multiple of 4")
                tA = make_tiles()
                tB = make_tiles()
                for p in a_parts(tA):
                    p()
                with tc.For_i(0, repeat // 4):
                    one_body(tA, a_parts(tB))
                    one_body(tB, a_parts(tA))
                    one_body(tA, a_parts(tB))
                    one_body(tB, a_parts(tA))
                return nc

            # ---- software-pipelined emission across unrolled bodies ----
            cur = make_tiles()
            for p in a_parts(cur):
                p()
            for rep in range(repeat):
                if rep + 1 < repeat:
                    nxt = make_tiles()
                    pending = a_parts(nxt)
                else:
                    nxt, pending = None, []
                one_body(cur, pending)
                cur = nxt

    return nc


def prepare_in_maps(x, W_qkv, b_qkv, W_out):
    """Host-side sharding/layout prep. Returns (in_maps, e_in)."""
    x = np.asarray(x, dtype=np.float32).reshape(L, E)
    W_qkv = np.asarray(W_qkv, dtype=np.float32)
    b_qkv = np.asarray(b_qkv, dtype=np.float32)
    W_out = np.asarray(W_out, dtype=np.float32)

    use_bias = bool(np.any(b_qkv))
    if use_bias:
        x_aug = np.concatenate([x, np.ones((L, 1), np.float32)], axis=1)
        W_aug = np.concatenate([W_qkv, b_qkv[None, :]], axis=0)
    else:
        x_aug, W_aug = x, W_qkv
    e_in = x_aug.shape[1]

    bf = ml_dtypes.bfloat16
    # partition-major: xT[p, c*L + l] = x_aug[l, c*128 + p]
    xTf = np.ascontiguousarray(x_aug.T).astype(bf)          # [e_in, L]
    xT = np.ascontiguousarray(
        xTf[: 4 * 128].reshape(4, 128, L).transpose(1, 0, 2).reshape(
            128, 4 * L))
    xTb = np.ascontiguousarray(xTf[4 * 128 :]) if e_in > 4 * 128 else None

    pos = np.arange(L, dtype=np.float32)
    theta = (np.pi / 2) * pos / L
    cosw = np.cos(theta).astype(np.float32)
    sinw = np.sin(theta).astype(np.float32)
    cosT = np.ascontiguousarray(cosw[None, :]).astype(bf)
    sinT = np.ascontiguousarray(sinw[None, :]).astype(bf)
    cosN = np.ascontiguousarray(cosw.reshape(NCH, C).T).astype(bf)
    sinN = np.ascontiguousarray(sinw.reshape(NCH, C).T).astype(bf)
    # mask[m, l] = cos(theta_l - theta_m) for m <= l else 0  (Toeplitz intra)
    lm = np.arange(C, dtype=np.float32)
    dth = (np.pi / 2) * (lm[None, :] - lm[:, None]) / L
    mask = (np.triu(np.ones((C, C), np.float32)) * np.cos(dth)).astype(bf)
    ident = np.eye(C, dtype=np.float32).astype(bf)

    in_maps = []
    for h in range(N_CORES):
        hs = slice(h * D, (h + 1) * D)
        wq_h = W_aug[:, hs]
        wk_h = W_aug[:, E + h * D : E + (h + 1) * D]
        wv_h = W_aug[:, 2 * E + h * D : 2 * E + (h + 1) * D]
        wqk_h = np.ascontiguousarray(
            np.concatenate([wq_h, wk_h], axis=1)).astype(bf)
        wkv_h = np.ascontiguousarray(
            np.concatenate([wk_h, wv_h], axis=1)).astype(bf)
        w2_h = W_out[hs, :]
        w2d_h = np.ascontiguousarray(
            np.concatenate([w2_h, w2_h], axis=0)).astype(bf)
        im = {
            "xT": xT, "wqk": wqk_h, "wkv": wkv_h, "w2d": w2d_h,
            "cosT": cosT, "sinT": sinT, "cosN": cosN, "sinN": sinN,
            "mask": mask, "ident": ident,
        }
        if xTb is not None:
            im["xTb"] = xTb
        in_maps.append(im)
    return in_maps, e_in


def combine_outputs(results, b_out):
    b_out = np.asarray(b_out, dtype=np.float32)
    accP = np.zeros((128, 2, 4, 1024), np.float32)
    for r in results:
        accP += np.asarray(r["outP"]).astype(np.float32)
    # outP[p, h2, n, l'] -> acc[n*128+p, h2*1024+l']
    acc = np.ascontiguousarray(
        accP.transpose(2, 0, 1, 3).reshape(E, L))
    # unpermute column slots -> chunk order
    out = np.empty((E, L), np.float32)
    for s, ch in enumerate(CHUNK_OF_SLOT):
        out[:, ch * C : (ch + 1) * C] = acc[:, s * C : (s + 1) * C]
    out = out.T + b_out[None, :]
    return out.reshape(B, L, E).astype(np.float32)


_PROGRAM_CACHE = {}


def _get_program(e_in):
    if e_in not in _PROGRAM_CACHE:
        _PROGRAM_CACHE[e_in] = build_program(e_in=e_in)
    return _PROGRAM_CACHE[e_in]


def kernel(x, W_qkv, b_qkv, W_out, b_out):
    from concourse.bass_utils import run_bass_kernel_spmd

    in_maps, e_in = prepare_in_maps(x, W_qkv, b_qkv, W_out)
    nc = _get_program(e_in)
    res = run_bass_kernel_spmd(nc, in_maps, core_ids=list(range(N_CORES)))
    return combine_outputs(res.results, b_out)



# revision 38
# speedup vs baseline: 1053.6740x; 1.2293x over previous
"""Cosformer causal attention (B=1, L=2048, E=512, H=8) on 8 TRN2 NeuronCores.

One head per core; trig folded into constants wherever the cosformer
algebra allows:
  A[l, m] = relu(q_l).relu(k_m) * cos(th_l - th_m)   (th = pi/2 * pos/L)

Per core (head h), from raw x (bf16, partition-major xT [128, 4*L],
one contiguous 16KB read per partition -> single input DMA):
  Q-stream : psum = [Wq|Wk]^T xT -> relu -> qk [128, L] (q rows 0-63,
             k' rows 64-127)
  scalings : qcqs = [q'*cos ; q'*sin] via 2 Pool ops + one DMA partition
             shift per L-half; the intra-chunk cos(th_l - th_m) is
             TOEPLITZ, folded into the causal mask constant.
  KV-stream: psum = [Wk|Wv]^T xT -> k' relu'd rows 0-63, v rows 64-127
             (kvt [128, L]); k' at partitions 0-63 doubles as the S0
             stationary operand (no shift). Per-chunk PAIRED PE
             transposes -> ktv [C, NCH, 129] = [k' | v | ones] chunk-row
             layout; kn = [k'*cos | k'*sin] (free-packed, Pool).
  P2 (C=128 chunks, groups of 4): s0 = k'^T q' ; at = s0 * maskcos (DVE);
             O = at^T [v|1] + qcqs^T kvsnap (K=128 packs cos+sin);
             KV state accumulates in a persistent PSUM bank (kn^T [v|1],
             one M=128-packed matmul/chunk) with per-chunk bf16 snapshots;
             the state chain is emitted ahead of each group so its latency
             overlaps the S0/intra work.
  P3: obf = O[:, :64]/(O[:, 64]+eps); PAIRED PE transposes -> otb
      [128, 8, C]; out-projection against w2dup (W_out head rows
      duplicated to both partition halves); psum -> bf16 staging (ACT/DVE
      split) -> one 1MB store per L-half (rings split via store_eng);
      outP column slots are chunk-permuted (CHUNK_OF_SLOT; host unpermutes).
Timing programs run `repeat` bodies inside a For_i HARDWARE loop
(constant static program size — per-call NEFF upload overhead, ~50-100us
per static instruction through the axon tunnel, is repeat-independent and
cancels in the test harness slope), `unroll` bodies per iteration,
software-pipelined: body i+1's DMA/projections interleave into body i's
P2 groups (in-order engines fill dependency stalls; the loop's all-engine
barrier is amortized 1/unroll).
Host: out = sum_h unperm(outP_h)^T + b_out.  rel err ~4.6e-3 vs reference.
"""

import numpy as np
import ml_dtypes

import concourse.bass as bass
import concourse.mybir as mybir
from concourse.tile import TileContext
from concourse.vector_clock import ScopedClock

BF16 = mybir.dt.bfloat16
F32 = mybir.dt.float32
AF = mybir.ActivationFunctionType
ALU = mybir.AluOpType

B, L, E, H = 1, 2048, 512, 8
D = E // H            # 64 head dim
C = 128               # chunk length
NCH = L // C          # 16 chunks
GRP = 4               # chunks per psum group
NG = NCH // GRP       # 4 groups
LT = 512              # l tile for N=512 matmul streams
NLT = L // LT         # 4
EPS = 1e-6
N_CORES = 8

# outP column slot s holds chunk CHUNK_OF_SLOT[s] (see P3 pairing)
CHUNK_OF_SLOT = [0, 2, 4, 6, 1, 3, 5, 7, 8, 10, 12, 14, 9, 11, 13, 15]
CHUNK_OF_SLOT_WIDE = [0, 2, 4, 6, 8, 10, 12, 14, 1, 3, 5, 7, 9, 11, 13, 15]
P3WIDE = False  # default out-proj flavor used by kernel()/combine_outputs


def _split_multi_waits(bir_json):
    """The walrus in this container accepts at most ONE sem wait per
    instruction; split extras into standalone EventSemaphore waits placed
    immediately before the instruction (same engine => order preserved)."""
    import json as _json

    js = _json.loads(bir_json)
    ctr = 0
    for fn in js.get("functions", []):
        for bb in fn.get("blocks", []):
            insts = bb.get("instructions")
            if not insts:
                continue
            out = []
            changed = False
            for inst in insts:
                si = inst.get("sync_info")
                waits = si.get("on_wait", []) if si else []
                if len(waits) > 1:
                    changed = True
                    for w in waits[:-1]:
                        ctr += 1
                        out.append({
                            "debug": inst.get("debug", 0),
                            "engine": inst["engine"],
                            "ins": [],
                            "name": f"I-splitw-{ctr}",
                            "opcode": "EventSemaphore",
                            "outs": [],
                            "sync_info": {"on_update": [], "on_wait": [w]},
                        })
                    si["on_wait"] = [waits[-1]]
                out.append(inst)
            if changed:
                bb["instructions"] = out
    return _json.dumps(js).encode()


def _install_wait_split_hook():
    import concourse.bass2jax as bass2jax
    import concourse.bass_utils as bass_utils

    if getattr(bass2jax, "_wait_split_installed", False):
        return
    orig = bass_utils.compile_bir_kernel

    def patched(bir_json, tmpdir, neff_name="file.neff"):
        return orig(_split_multi_waits(bir_json), tmpdir, neff_name=neff_name)

    bass2jax.compile_bir_kernel = patched
    bass_utils.compile_bir_kernel = patched
    bass2jax._wait_split_installed = True


_install_wait_split_hook()


class SplitDrainTileContext(TileContext):
    """walrus in this container rejects >1 sem wait on the final SP Drain;
    spread the accumulated waits over single-wait SP wait instructions."""

    def _drain_and_barrier(self, tick_clock, wait_clock):
        nc = self.nc
        drain_inst = nc.sync.drain()
        wait_clock.add_sem_waits(
            drain_inst.ins, ScopedClock({None: tick_clock.global_clock})
        )
        si = drain_inst.ins.sync_info
        waits = list(si.on_wait) if si is not None else []
        if len(waits) > 1:
            drain_inst.ins.sync_info.on_wait = waits[:1]
            name2sem = {v.name: v for v in self.sems.allocated().values()}
            for w in waits[1:]:
                nc.sync.wait_ge(name2sem[w.ant_name], w.wait_value)
        nc.all_engine_barrier()
        popped = nc._tile_sem_poison_stack.pop()
        assert popped is self._sem_poison
        nc.clear_and_free_semaphores(list(self.sems.allocated().values()))
        nc.all_engine_barrier()


def build_program(e_in=E, repeat=1, debug=False, hw_loop=False, ablate=(),
                  store_eng="split", unroll=4, state2=False, p3wide=None,
                  balance="v1", p3tile=False):
    """hw_loop=True runs `repeat` bodies via a For_i hardware loop (constant
    program size); hw_loop=False unrolls them (legacy). `ablate` names body
    components to omit — timing-probe builds only, output is garbage."""
    abl = set(ablate)
    if p3wide is None:
        p3wide = P3WIDE
    nc = bass.Bass("TRN2", target_bir_lowering=False, debug=False,
                   num_devices=N_CORES)

    ecs = [(i, 128) for i in range(4)]
    if e_in > 4 * 128:
        assert e_in == 4 * 128 + 1
        ecs.append((4, e_in - 4 * 128))
    NEC = len(ecs)

    # x transposed, partition-major: xT[p, c*L + l] = x_aug[l, c*128 + p]
    # (one contiguous 16KB read per partition for the body's single x DMA)
    xT = nc.dram_tensor("xT", [128, 4 * L], BF16, kind="ExternalInput")
    if e_in > 4 * 128:
        assert e_in == 4 * 128 + 1
        xTb = nc.dram_tensor("xTb", [1, L], BF16, kind="ExternalInput")
    wqk = nc.dram_tensor("wqk", [e_in, 128], BF16, kind="ExternalInput")
    wkv = nc.dram_tensor("wkv", [e_in, 128], BF16, kind="ExternalInput")
    w2d = nc.dram_tensor("w2d", [128, E], BF16, kind="ExternalInput")
    cosTd = nc.dram_tensor("cosT", [1, L], BF16, kind="ExternalInput")
    sinTd = nc.dram_tensor("sinT", [1, L], BF16, kind="ExternalInput")
    cosNd = nc.dram_tensor("cosN", [C, NCH], BF16, kind="ExternalInput")
    sinNd = nc.dram_tensor("sinN", [C, NCH], BF16, kind="ExternalInput")
    maskd = nc.dram_tensor("mask", [C, C], BF16, kind="ExternalInput")
    identd = nc.dram_tensor("ident", [C, C], BF16, kind="ExternalInput")
    # outP[p, h2, n, l'] = column-slot-permuted out[n*128 + p, h2*1024 + l']
    # (one contiguous 8KB write per partition per h2-half store DMA)
    outP = nc.dram_tensor("outP", [128, 2, 4, 1024], BF16,
                          kind="ExternalOutput")

    with SplitDrainTileContext(nc) as tc:
        with (
            tc.tile_pool(name="const", bufs=1) as cpool,
            tc.tile_pool(name="work", bufs=2) as wpool,
            tc.tile_pool(name="stage", bufs=3) as spool,
            tc.tile_pool(name="pbig", bufs=2, space="PSUM") as pbig,
            tc.tile_pool(name="pso", bufs=2, space="PSUM") as pso,
            tc.tile_pool(name="pkv", bufs=2, space="PSUM") as pkv,
        ):
            # ---- constants (one slot, DMA'd once) ----
            wqk_sb = cpool.tile([128, NEC, 128], BF16, tag="wqk")
            wkv_sb = cpool.tile([128, NEC, 128], BF16, tag="wkv")
            w2d_sb = cpool.tile([128, NLT, 128], BF16, tag="w2d")
            cosT_sb = cpool.tile([128, L], BF16, tag="cosT")
            sinT_sb = cpool.tile([128, L], BF16, tag="sinT")
            cosN_sb = cpool.tile([C, NCH], BF16, tag="cosN")
            sinN_sb = cpool.tile([C, NCH], BF16, tag="sinN")
            mask_sb = cpool.tile([C, C], BF16, tag="mask")
            ident_sb = cpool.tile([C, C], BF16, tag="ident")

            for t_sb, t_d in [(wqk_sb, wqk), (wkv_sb, wkv)]:
                nc.sync.dma_start(
                    t_sb[:, :4, :],
                    t_d[: 4 * 128, :].rearrange("(c p) d -> p c d", p=128),
                )
                if NEC == 5:
                    nc.sync.dma_start(t_sb[:1, 4, :], t_d[4 * 128 :, :])
            nc.sync.dma_start(
                cosT_sb[:], cosTd[:1, :].to_broadcast([128, L]))
            nc.sync.dma_start(
                sinT_sb[:], sinTd[:1, :].to_broadcast([128, L]))
            nc.sync.dma_start(
                w2d_sb[:], w2d.rearrange("d (t n) -> d t n", n=128)
            )
            nc.sync.dma_start(cosN_sb[:], cosNd[:])
            nc.sync.dma_start(sinN_sb[:], sinNd[:])
            nc.sync.dma_start(mask_sb[:], maskd[:])
            nc.sync.dma_start(ident_sb[:], identd[:])

            def make_tiles():
                tl = {}
                tl["xT"] = wpool.tile([128, NEC, L], BF16, tag="xT", name="xTs")
                tl["qk"] = wpool.tile([128, L], BF16, tag="qk", name="qkt")
                tl["qcqs"] = wpool.tile([128, L], BF16, tag="qcqs", name="qcqst")
                tl["qstmp"] = wpool.tile([D, L], BF16, tag="qstmp", name="qstmpt")
                tl["kvt"] = wpool.tile([128, L], BF16, tag="kvt", name="kvtt")
                tl["kn"] = wpool.tile([C, NCH, 128], BF16, tag="kn", name="knt")
                tl["ktv"] = wpool.tile([C, NCH, 129], BF16, tag="ktv", name="ktvt")
                tl["at"] = wpool.tile([C, NCH, C], BF16, tag="at", name="att")
                tl["kvs"] = wpool.tile([128, NCH - 1, D + 1], BF16, tag="kvs", name="kvst")
                if state2:
                    tl["kvsB"] = wpool.tile([128, 7, D + 1], BF16,
                                            tag="kvsB", name="kvsBt")
                tl["obf"] = wpool.tile([C, NCH, D], BF16, tag="obf", name="obft")
                tl["otb"] = wpool.tile([128, NCH // 2, C], BF16, tag="otb", name="otbt")
                tl["rtmp"] = wpool.tile([C, NCH], F32, tag="rtmp", name="rtmpt")
                tl["r"] = wpool.tile([C, NCH], F32, tag="r", name="rt")
                return tl

            def a_parts(tl):
                """Input DMA + projections + scalings for one body, as a list
                of emission thunks (interleaved into the previous body)."""
                parts = []

                def dma_x(eng):
                    def f():
                        if "xdma" in abl:
                            nc.gpsimd.memset(
                                tl["ktv"][:, :, 128:129], 1.0)
                            return
                        eng.dma_start(
                            tl["xT"][:, :4, :],
                            xT[:, :].rearrange("p (c l) -> p c l", l=L),
                        )
                        if NEC == 5:
                            eng.dma_start(tl["xT"][:1, 4, :], xTb[:, :])
                        nc.gpsimd.memset(tl["ktv"][:, :, 128:129], 1.0)
                    return f

                def tproj(tp):
                    def f():
                        if "tstream" in abl:
                            return
                        pT = pbig.tile([128, 2, LT], F32, tag="big",
                                       name="pT")
                        for half in range(2):
                            t = 2 * tp + half
                            ls = slice(t * LT, (t + 1) * LT)
                            for i, (ec, pc) in enumerate(ecs):
                                nc.tensor.matmul(
                                    pT[:, half, :], wqk_sb[:pc, ec, :],
                                    tl["xT"][:pc, ec, ls],
                                    start=(i == 0), stop=(i == NEC - 1),
                                )
                        ls2 = slice(tp * 2 * LT, (tp + 1) * 2 * LT)
                        if tp == 0:
                            nc.scalar.activation(tl["qk"][:, ls2], pT[:],
                                                 AF.Relu)
                        else:
                            nc.vector.tensor_scalar_max(tl["qk"][:, ls2],
                                                        pT[:], 0.0)
                    return f

                def scal(hl):
                    def f():
                        if "scal" in abl:
                            return
                        ls = slice(hl * (L // 2), (hl + 1) * (L // 2))
                        qeng = nc.gpsimd if balance == "v2" else nc.vector
                        qeng.tensor_tensor(
                            tl["qcqs"][:D, ls], tl["qk"][:D, ls],
                            cosT_sb[:D, ls], ALU.mult,
                        )
                        qeng.tensor_tensor(
                            tl["qstmp"][:, ls], tl["qk"][:D, ls],
                            sinT_sb[:D, ls], ALU.mult,
                        )
                        if "shift" not in abl:
                            nc.scalar.dma_start(tl["qcqs"][D:, ls],
                                                tl["qstmp"][:, ls])
                    return f

                def kvtp(tp):
                    # second T-stream: kvt = [Wk|Wv]^T xT; k rows relu'd
                    def f():
                        if "nstream" in abl:
                            return
                        pT = pbig.tile([128, 2, LT], F32, tag="big",
                                       name="pKV")
                        for half in range(2):
                            t = 2 * tp + half
                            ls = slice(t * LT, (t + 1) * LT)
                            for i, (ec, pc) in enumerate(ecs):
                                nc.tensor.matmul(
                                    pT[:, half, :], wkv_sb[:pc, ec, :],
                                    tl["xT"][:pc, ec, ls],
                                    start=(i == 0), stop=(i == NEC - 1),
                                )
                        ls2 = slice(tp * 2 * LT, (tp + 1) * 2 * LT)
                        nc.scalar.activation(tl["kvt"][:D, ls2],
                                             pT[:D, :, :], AF.Relu)
                        nc.vector.tensor_copy(tl["kvt"][D:, ls2],
                                              pT[D:, :, :])
                    return f

                def ktr(h):
                    # paired transposes: chunk-row layout ktv[l, j, [k'|v]]
                    def f():
                        if "nstream" in abl:
                            return
                        gs = slice(h * 2 * GRP, (h + 1) * 2 * GRP)
                        ptr = pso.tile([C, 2 * GRP, 128], BF16, tag="so",
                                       name="ptr")
                        for sub in range(2 * GRP):
                            j = h * 2 * GRP + sub
                            cs = slice(j * C, (j + 1) * C)
                            nc.tensor.transpose(
                                ptr[:, sub, :], tl["kvt"][:, cs],
                                ident_sb[:],
                            )
                        if h == 0:
                            nc.scalar.activation(tl["ktv"][:, gs, :128],
                                                 ptr[:], AF.Copy)
                        else:
                            nc.vector.tensor_copy(tl["ktv"][:, gs, :128],
                                                  ptr[:])
                        nc.gpsimd.tensor_tensor(
                            tl["kn"][:, gs, :D], tl["ktv"][:, gs, :D],
                            cosN_sb[:, gs, None].to_broadcast(
                                [C, 2 * GRP, D]),
                            ALU.mult,
                        )
                        nc.gpsimd.tensor_tensor(
                            tl["kn"][:, gs, D:], tl["ktv"][:, gs, :D],
                            sinN_sb[:, gs, None].to_broadcast(
                                [C, 2 * GRP, D]),
                            ALU.mult,
                        )
                    return f

                # HWDGE engines only (sync/scalar): a gpsimd (SWDGE) DMA in
                # a For_i body makes the loop reset emit INC_SWDGE_SEM,
                # which this container's walrus cannot encode.
                parts.append(dma_x(nc.sync))
                parts.append(tproj(0))
                parts.append(scal(0))
                parts.append(tproj(1))
                parts.append(scal(1))
                parts.append(kvtp(0))
                parts.append(ktr(0))
                parts.append(kvtp(1))
                parts.append(ktr(1))
                return parts

            def state_steps(tl, kv_ps, g):
                # kv accumulation + snapshots for chunks of group g (emitted
                # ahead of the group's O work; chain overlaps S0/intra phase)
                if "state" in abl:
                    return
                for sub in range(GRP):
                    j = g * GRP + sub
                    if j < NCH - 1:
                        nc.tensor.matmul(
                            kv_ps[:, : D + 1], tl["kn"][:, j, :],
                            tl["ktv"][:, j, D:],
                            start=(j == 0), stop=(j == NCH - 2),
                            skip_group_check=True,
                        )
                        if j % 2 == 0:
                            nc.scalar.activation(
                                tl["kvs"][:, j, :], kv_ps[:, : D + 1],
                                AF.Copy)
                        else:
                            nc.vector.tensor_copy(
                                tl["kvs"][:, j, :], kv_ps[:, : D + 1])

            def state_steps2(tl, kv_psA, kv_psB, g):
                # two parallel prefix chains (chunks 0-7 / 8-14), both done
                # by end of group 1; halves the serial mm->snapshot latency.
                # kvs[j>=8] gets chain-A's total added in one batched DVE op
                # (one extra bf16 rounding vs the single-chain version).
                if "state" in abl or g >= 2:
                    return
                for st in range(GRP):
                    jA = g * GRP + st
                    nc.tensor.matmul(
                        kv_psA[:, : D + 1], tl["kn"][:, jA, :],
                        tl["ktv"][:, jA, D:],
                        start=(jA == 0), stop=(jA == 7),
                        skip_group_check=True,
                    )
                    nc.scalar.activation(
                        tl["kvs"][:, jA, :], kv_psA[:, : D + 1], AF.Copy)
                    jB = 8 + g * GRP + st
                    if jB <= 14:
                        nc.tensor.matmul(
                            kv_psB[:, : D + 1], tl["kn"][:, jB, :],
                            tl["ktv"][:, jB, D:],
                            start=(jB == 8), stop=(jB == 14),
                            skip_group_check=True,
                        )
                        nc.vector.tensor_copy(
                            tl["kvsB"][:, jB - 8, :], kv_psB[:, : D + 1])
                if g == 1:
                    nc.vector.tensor_tensor(
                        tl["kvs"][:, 8:15, :], tl["kvsB"][:],
                        tl["kvs"][:, 7, None, :].to_broadcast(
                            [128, 7, D + 1]),
                        ALU.add,
                    )

            def b_group(tl, g):
                gs = slice(g * GRP, (g + 1) * GRP)
                if "s0" not in abl:
                    b_group_s0(tl, g, gs)
                if "ov" not in abl:
                    b_group_ov(tl, g, gs)

            def b_group_s0(tl, g, gs):
                s0 = pso.tile([C, GRP, C], F32, tag="so")
                for sub in range(GRP):
                    j = g * GRP + sub
                    cs = slice(j * C, (j + 1) * C)
                    nc.tensor.matmul(
                        s0[:, sub, :], tl["kvt"][:D, cs], tl["qk"][:D, cs],
                        start=True, stop=True,
                    )
                nc.vector.tensor_tensor(
                    tl["at"][:, gs, :], s0[:],
                    mask_sb[:, None, :].to_broadcast([C, GRP, C]), ALU.mult,
                )

            def b_group_ov(tl, g, gs):
                o_ps = pso.tile([C, GRP, 128], F32, tag="so")
                for sub in range(GRP):
                    j = g * GRP + sub
                    cs = slice(j * C, (j + 1) * C)
                    nc.tensor.matmul(
                        o_ps[:, sub, : D + 1], tl["at"][:, j, :],
                        tl["ktv"][:, j, D:],
                        start=True, stop=(j == 0),
                    )
                    if j > 0:
                        nc.tensor.matmul(
                            o_ps[:, sub, : D + 1], tl["qcqs"][:, cs],
                            tl["kvs"][:, j - 1, :],
                            start=False, stop=True,
                        )
                if "norm" in abl:
                    nc.scalar.activation(tl["obf"][:, gs, :],
                                         o_ps[:, :, :D], AF.Copy)
                    return
                nc.vector.tensor_scalar_add(
                    tl["rtmp"][:, gs], o_ps[:, :, D], EPS)
                nc.vector.reciprocal(tl["r"][:, gs], tl["rtmp"][:, gs])
                nc.vector.tensor_tensor(
                    tl["obf"][:, gs, :], o_ps[:, :, :D],
                    tl["r"][:, gs, None].to_broadcast([C, GRP, D]), ALU.mult,
                )

            def c_tr(tl, h2):
                if "ctr" in abl:
                    return
                tp = pso.tile([128, NCH // 4, C], BF16, tag="so", name="tp")
                for tt in range(NCH // 4):
                    t = h2 * (NCH // 4) + tt
                    nc.tensor.transpose(
                        tp[:, tt, :], tl["obf"][:, 2 * t : 2 * t + 2, :],
                        ident_sb[:],
                    )
                ts = slice(h2 * 4, (h2 + 1) * 4)
                if h2 == 0 or balance == "v1":
                    nc.vector.tensor_copy(tl["otb"][:, ts, :], tp[:])
                else:
                    nc.scalar.activation(tl["otb"][:, ts, :], tp[:], AF.Copy)

            def c_p3(tl, h2, nss):
                if "oproj" in abl:
                    return
                ts = slice(h2 * 4, (h2 + 1) * 4)
                ob = spool.tile([128, 4, 2, LT], BF16, tag="ob")
                for ns in nss:
                    pab = pbig.tile([128, 2, LT], F32, tag="big", name="pab")
                    tp0 = (0, 0) if p3tile else None
                    tp1 = (64, 0) if p3tile else None
                    nc.tensor.matmul(
                        pab[:, 0, :], w2d_sb[:D, ns, :], tl["otb"][:D, ts, :],
                        start=True, stop=True, tile_position=tp0,
                    )
                    nc.tensor.matmul(
                        pab[:, 1, :], w2d_sb[D:, ns, :], tl["otb"][D:, ts, :],
                        start=True, stop=True, tile_position=tp1,
                    )
                    dve = (ns % 2 == 1) if balance == "v2" \
                        else ((h2 * NLT + ns) % 4 == 1)
                    if dve:
                        nc.vector.tensor_copy(ob[:, ns], pab[:])
                    else:
                        nc.scalar.activation(ob[:, ns], pab[:], AF.Copy)
                if "store" not in abl:
                    if store_eng == "split":
                        seng = nc.scalar if h2 == 0 else nc.sync
                    else:
                        seng = nc.sync if store_eng == "sync" else nc.scalar
                    seng.dma_start(
                        outP[:, h2, :, :],
                        ob[:].rearrange("p n a b -> p n (a b)"),
                    )

            def one_body(tl, pending=()):
                """Emit one body's compute, interleaving `pending` emission
                thunks (the next body's input DMA/projections) into the
                group structure so in-order engines fill dependency stalls."""
                pi = iter(pending)

                def drain(n):
                    for _ in range(n):
                        p = next(pi, None)
                        if p is not None:
                            p()

                if state2:
                    kv_psA = pkv.tile([128, D + 1], F32, tag="kv")
                    kv_psB = pkv.tile([128, D + 1], F32, tag="kv")
                else:
                    kv_ps = pkv.tile([128, D + 1], F32, tag="kv")
                for g in range(NG):
                    if state2:
                        state_steps2(tl, kv_psA, kv_psB, g)
                    else:
                        state_steps(tl, kv_ps, g)
                    b_group(tl, g)
                    if g == 1:
                        c_tr(tl, 0)
                        if not p3wide:
                            c_p3(tl, 0, [0, 1, 2, 3])
                    drain(2)
                c_tr(tl, 1)
                if p3wide:
                    c_p3w(tl)
                else:
                    c_p3(tl, 1, [0, 1, 2, 3])
                for p in pi:
                    p()

            if hw_loop:
                if repeat == 1:
                    cur = make_tiles()
                    for p in a_parts(cur):
                        p()
                    one_body(cur)
                    return nc
                assert repeat % unroll == 0, (
                    "hw_loop repeat must be 1 or a multiple of unroll")
                tA = make_tiles()
                tB = make_tiles()
                for p in a_parts(tA):
                    p()
                with tc.For_i(0, repeat // unroll):
                    for u in range(unroll // 2):
                        one_body(tA, a_parts(tB))
                        one_body(tB, a_parts(tA))
                return nc

            def c_p3w(tl):
                # out-proj as 8 single-shot N=1024 matmuls (bf16 psum out);
                # slot order: 0-7 = even chunks (otb top), 8-15 = odd (bottom)
                if "oproj" in abl:
                    return
                obs = []
                for half in range(2):
                    ob = spool.tile([128, 4, 2, LT], BF16, tag="ob")
                    hs = slice(0, D) if half == 0 else slice(D, 128)
                    for ns in range(4):
                        pab = pbig.tile([128, 2 * GRP, C], F32, tag="big",
                                        name="pabw")
                        nc.tensor.matmul(
                            pab[:], w2d_sb[hs, ns, :], tl["otb"][hs, :, :],
                            start=True, stop=True,
                        )
                        if (2 * half + ns) % 2 == 0:
                            nc.scalar.activation(
                                ob[:, ns].rearrange("p a b -> p (a b)"),
                                pab[:].rearrange("p s c -> p (s c)"),
                                AF.Copy)
                        else:
                            nc.vector.tensor_copy(
                                ob[:, ns].rearrange("p a b -> p (a b)"),
                                pab[:].rearrange("p s c -> p (s c)"))
                    obs.append(ob)
                for half in range(2):
                    seng = nc.sync if store_eng == "sync" else nc.scalar
                    seng.dma_start(
                        outP[:, half, :, :],
                        obs[half][:].rearrange("p n a b -> p n (a b)"),
                    )

            # ---- software-pipelined emission across unrolled bodies ----
            cur = make_tiles()
            for p in a_parts(cur):
                p()
            for rep in range(repeat):
                if rep + 1 < repeat:
                    nxt = make_tiles()
                    pending = a_parts(nxt)
                else:
                    nxt, pending = None, []
                one_body(cur, pending)
                cur = nxt

    return nc


def prepare_in_maps(x, W_qkv, b_qkv, W_out):
    """Host-side sharding/layout prep. Returns (in_maps, e_in)."""
    x = np.asarray(x, dtype=np.float32).reshape(L, E)
    W_qkv = np.asarray(W_qkv, dtype=np.float32)
    b_qkv = np.asarray(b_qkv, dtype=np.float32)
    W_out = np.asarray(W_out, dtype=np.float32)

    use_bias = bool(np.any(b_qkv))
    if use_bias:
        x_aug = np.concatenate([x, np.ones((L, 1), np.float32)], axis=1)
        W_aug = np.concatenate([W_qkv, b_qkv[None, :]], axis=0)
    else:
        x_aug, W_aug = x, W_qkv
    e_in = x_aug.shape[1]

    bf = ml_dtypes.bfloat16
    # partition-major: xT[p, c*L + l] = x_aug[l, c*128 + p]
    xTf = np.ascontiguousarray(x_aug.T).astype(bf)          # [e_in, L]
    xT = np.ascontiguousarray(
        xTf[: 4 * 128].reshape(4, 128, L).transpose(1, 0, 2).reshape(
            128, 4 * L))
    xTb = np.ascontiguousarray(xTf[4 * 128 :]) if e_in > 4 * 128 else None

    pos = np.arange(L, dtype=np.float32)
    theta = (np.pi / 2) * pos / L
    cosw = np.cos(theta).astype(np.float32)
    sinw = np.sin(theta).astype(np.float32)
    cosT = np.ascontiguousarray(cosw[None, :]).astype(bf)
    sinT = np.ascontiguousarray(sinw[None, :]).astype(bf)
    cosN = np.ascontiguousarray(cosw.reshape(NCH, C).T).astype(bf)
    sinN = np.ascontiguousarray(sinw.reshape(NCH, C).T).astype(bf)
    # mask[m, l] = cos(theta_l - theta_m) for m <= l else 0  (Toeplitz intra)
    lm = np.arange(C, dtype=np.float32)
    dth = (np.pi / 2) * (lm[None, :] - lm[:, None]) / L
    mask = (np.triu(np.ones((C, C), np.float32)) * np.cos(dth)).astype(bf)
    ident = np.eye(C, dtype=np.float32).astype(bf)

    in_maps = []
    for h in range(N_CORES):
        hs = slice(h * D, (h + 1) * D)
        wq_h = W_aug[:, hs]
        wk_h = W_aug[:, E + h * D : E + (h + 1) * D]
        wv_h = W_aug[:, 2 * E + h * D : 2 * E + (h + 1) * D]
        wqk_h = np.ascontiguousarray(
            np.concatenate([wq_h, wk_h], axis=1)).astype(bf)
        wkv_h = np.ascontiguousarray(
            np.concatenate([wk_h, wv_h], axis=1)).astype(bf)
        w2_h = W_out[hs, :]
        w2d_h = np.ascontiguousarray(
            np.concatenate([w2_h, w2_h], axis=0)).astype(bf)
        im = {
            "xT": xT, "wqk": wqk_h, "wkv": wkv_h, "w2d": w2d_h,
            "cosT": cosT, "sinT": sinT, "cosN": cosN, "sinN": sinN,
            "mask": mask, "ident": ident,
        }
        if xTb is not None:
            im["xTb"] = xTb
        in_maps.append(im)
    return in_maps, e_in


def combine_outputs(results, b_out, p3wide=None):
    if p3wide is None:
        p3wide = P3WIDE
    slot_order = CHUNK_OF_SLOT_WIDE if p3wide else CHUNK_OF_SLOT
    b_out = np.asarray(b_out, dtype=np.float32)
    accP = np.zeros((128, 2, 4, 1024), np.float32)
    for r in results:
        accP += np.asarray(r["outP"]).astype(np.float32)
    # outP[p, h2, n, l'] -> acc[n*128+p, h2*1024+l']
    acc = np.ascontiguousarray(
        accP.transpose(2, 0, 1, 3).reshape(E, L))
    # unpermute column slots -> chunk order
    out = np.empty((E, L), np.float32)
    for s, ch in enumerate(slot_order):
        out[:, ch * C : (ch + 1) * C] = acc[:, s * C : (s + 1) * C]
    out = out.T + b_out[None, :]
    return out.reshape(B, L, E).astype(np.float32)


_PROGRAM_CACHE = {}


def _get_program(e_in):
    if e_in not in _PROGRAM_CACHE:
        _PROGRAM_CACHE[e_in] = build_program(e_in=e_in)
    return _PROGRAM_CACHE[e_in]


def kernel(x, W_qkv, b_qkv, W_out, b_out):
    from concourse.bass_utils import run_bass_kernel_spmd

    in_maps, e_in = prepare_in_maps(x, W_qkv, b_qkv, W_out)
    nc = _get_program(e_in)
    res = run_bass_kernel_spmd(nc, in_maps, core_ids=list(range(N_CORES)))
    return combine_outputs(res.results, b_out)



# revision 39
# speedup vs baseline: 1472.4184x; 1.3974x over previous
"""Cosformer causal attention (B=1, L=2048, E=512, H=8) on 8 TRN2 NeuronCores.

One head per core; trig folded into constants wherever the cosformer
algebra allows:
  A[l, m] = relu(q_l).relu(k_m) * cos(th_l - th_m)   (th = pi/2 * pos/L)

Per core (head h), from raw x (bf16, partition-major xT [128, 4*L],
one contiguous 16KB read per partition -> single input DMA):
  Q-stream : psum = [Wq|Wk]^T xT -> relu -> qk [128, L] (q rows 0-63,
             k' rows 64-127)
  scalings : qcqs = [q'*cos ; q'*sin] via 2 Pool ops + one DMA partition
             shift per L-half; the intra-chunk cos(th_l - th_m) is
             TOEPLITZ, folded into the causal mask constant.
  KV-stream: psum = [Wk|Wv]^T xT -> k' relu'd rows 0-63, v rows 64-127
             (kvt [128, L]); k' at partitions 0-63 doubles as the S0
             stationary operand (no shift). Per-chunk PAIRED PE
             transposes -> ktv [C, NCH, 129] = [k' | v | ones] chunk-row
             layout; kn = [k'*cos | k'*sin] (free-packed, Pool).
  P2 (C=128 chunks, groups of 4): s0 = k'^T q' ; at = s0 * maskcos (DVE);
             O = at^T [v|1] + qcqs^T kvsnap (K=128 packs cos+sin);
             KV state accumulates in a persistent PSUM bank (kn^T [v|1],
             one M=128-packed matmul/chunk) with per-chunk bf16 snapshots;
             the state chain is emitted ahead of each group so its latency
             overlaps the S0/intra work.
  P3: obf = O[:, :64]/(O[:, 64]+eps); PAIRED PE transposes -> otb
      [128, 8, C]; out-projection against w2dup (W_out head rows
      duplicated to both partition halves); psum -> bf16 staging (ACT/DVE
      split) -> one 1MB store per L-half (rings split via store_eng);
      outP column slots are chunk-permuted (CHUNK_OF_SLOT; host unpermutes).
Timing programs run `repeat` bodies inside a For_i HARDWARE loop
(constant static program size — per-call NEFF upload overhead, ~50-100us
per static instruction through the axon tunnel, is repeat-independent and
cancels in the test harness slope), `unroll` bodies per iteration,
software-pipelined: body i+1's DMA/projections interleave into body i's
P2 groups (in-order engines fill dependency stalls; the loop's all-engine
barrier is amortized 1/unroll).
Host: out = sum_h unperm(outP_h)^T + b_out.  rel err ~4.6e-3 vs reference.
"""

import numpy as np
import ml_dtypes

import concourse.bass as bass
import concourse.mybir as mybir
from concourse.tile import TileContext
from concourse.vector_clock import ScopedClock

BF16 = mybir.dt.bfloat16
F32 = mybir.dt.float32
AF = mybir.ActivationFunctionType
ALU = mybir.AluOpType

B, L, E, H = 1, 2048, 512, 8
D = E // H            # 64 head dim
C = 128               # chunk length
NCH = L // C          # 16 chunks
GRP = 4               # chunks per psum group
NG = NCH // GRP       # 4 groups
LT = 512              # l tile for N=512 matmul streams
NLT = L // LT         # 4
EPS = 1e-6
N_CORES = 8

# outP column slot s holds chunk CHUNK_OF_SLOT[s] (see P3 pairing)
CHUNK_OF_SLOT = [0, 2, 4, 6, 1, 3, 5, 7, 8, 10, 12, 14, 9, 11, 13, 15]
CHUNK_OF_SLOT_WIDE = [0, 2, 4, 6, 8, 10, 12, 14, 1, 3, 5, 7, 9, 11, 13, 15]
P3WIDE = False  # default out-proj flavor used by kernel()/combine_outputs


def _split_multi_waits(bir_json):
    """The walrus in this container accepts at most ONE sem wait per
    instruction; split extras into standalone EventSemaphore waits placed
    immediately before the instruction (same engine => order preserved)."""
    import json as _json

    js = _json.loads(bir_json)
    ctr = 0
    for fn in js.get("functions", []):
        for bb in fn.get("blocks", []):
            insts = bb.get("instructions")
            if not insts:
                continue
            out = []
            changed = False
            for inst in insts:
                si = inst.get("sync_info")
                waits = si.get("on_wait", []) if si else []
                if len(waits) > 1:
                    changed = True
                    for w in waits[:-1]:
                        ctr += 1
                        out.append({
                            "debug": inst.get("debug", 0),
                            "engine": inst["engine"],
                            "ins": [],
                            "name": f"I-splitw-{ctr}",
                            "opcode": "EventSemaphore",
                            "outs": [],
                            "sync_info": {"on_update": [], "on_wait": [w]},
                        })
                    si["on_wait"] = [waits[-1]]
                out.append(inst)
            if changed:
                bb["instructions"] = out
    return _json.dumps(js).encode()


def _install_wait_split_hook():
    import concourse.bass2jax as bass2jax
    import concourse.bass_utils as bass_utils

    if getattr(bass2jax, "_wait_split_installed", False):
        return
    orig = bass_utils.compile_bir_kernel

    def patched(bir_json, tmpdir, neff_name="file.neff"):
        return orig(_split_multi_waits(bir_json), tmpdir, neff_name=neff_name)

    bass2jax.compile_bir_kernel = patched
    bass_utils.compile_bir_kernel = patched
    bass2jax._wait_split_installed = True


_install_wait_split_hook()


class SplitDrainTileContext(TileContext):
    """walrus in this container rejects >1 sem wait on the final SP Drain;
    spread the accumulated waits over single-wait SP wait instructions."""

    def _drain_and_barrier(self, tick_clock, wait_clock):
        nc = self.nc
        drain_inst = nc.sync.drain()
        wait_clock.add_sem_waits(
            drain_inst.ins, ScopedClock({None: tick_clock.global_clock})
        )
        si = drain_inst.ins.sync_info
        waits = list(si.on_wait) if si is not None else []
        if len(waits) > 1:
            drain_inst.ins.sync_info.on_wait = waits[:1]
            name2sem = {v.name: v for v in self.sems.allocated().values()}
            for w in waits[1:]:
                nc.sync.wait_ge(name2sem[w.ant_name], w.wait_value)
        nc.all_engine_barrier()
        popped = nc._tile_sem_poison_stack.pop()
        assert popped is self._sem_poison
        nc.clear_and_free_semaphores(list(self.sems.allocated().values()))
        nc.all_engine_barrier()


def build_program(e_in=E, repeat=1, debug=False, hw_loop=False, ablate=(),
                  store_eng="split", unroll=4, state2=False, p3wide=None,
                  balance="v1", p3tile=False, xsplit=False):
    """hw_loop=True runs `repeat` bodies via a For_i hardware loop (constant
    program size); hw_loop=False unrolls them (legacy). `ablate` names body
    components to omit — timing-probe builds only, output is garbage."""
    abl = set(ablate)
    if p3wide is None:
        p3wide = P3WIDE
    nc = bass.Bass("TRN2", target_bir_lowering=False, debug=False,
                   num_devices=N_CORES)

    ecs = [(i, 128) for i in range(4)]
    if e_in > 4 * 128:
        assert e_in == 4 * 128 + 1
        ecs.append((4, e_in - 4 * 128))
    NEC = len(ecs)

    # x transposed, partition-major: xT[p, c*L + l] = x_aug[l, c*128 + p]
    # (one contiguous 16KB read per partition for the body's single x DMA)
    xT = nc.dram_tensor("xT", [128, 4 * L], BF16, kind="ExternalInput")
    if e_in > 4 * 128:
        assert e_in == 4 * 128 + 1
        xTb = nc.dram_tensor("xTb", [1, L], BF16, kind="ExternalInput")
    wqk = nc.dram_tensor("wqk", [e_in, 128], BF16, kind="ExternalInput")
    wkv = nc.dram_tensor("wkv", [e_in, 128], BF16, kind="ExternalInput")
    w2d = nc.dram_tensor("w2d", [128, E], BF16, kind="ExternalInput")
    cosTd = nc.dram_tensor("cosT", [1, L], BF16, kind="ExternalInput")
    sinTd = nc.dram_tensor("sinT", [1, L], BF16, kind="ExternalInput")
    cosNd = nc.dram_tensor("cosN", [C, NCH], BF16, kind="ExternalInput")
    sinNd = nc.dram_tensor("sinN", [C, NCH], BF16, kind="ExternalInput")
    maskd = nc.dram_tensor("mask", [C, C], BF16, kind="ExternalInput")
    identd = nc.dram_tensor("ident", [C, C], BF16, kind="ExternalInput")
    # outP[p, h2, n, l'] = column-slot-permuted out[n*128 + p, h2*1024 + l']
    # (one contiguous 8KB write per partition per h2-half store DMA)
    outP = nc.dram_tensor("outP", [128, 2, 4, 1024], BF16,
                          kind="ExternalOutput")

    with SplitDrainTileContext(nc) as tc:
        with (
            tc.tile_pool(name="const", bufs=1) as cpool,
            tc.tile_pool(name="work", bufs=2) as wpool,
            tc.tile_pool(name="stage", bufs=3) as spool,
            tc.tile_pool(name="pbig", bufs=2, space="PSUM") as pbig,
            tc.tile_pool(name="pso", bufs=2, space="PSUM") as pso,
            tc.tile_pool(name="pkv", bufs=2, space="PSUM") as pkv,
        ):
            # ---- constants (one slot, DMA'd once) ----
            wqk_sb = cpool.tile([128, NEC, 128], BF16, tag="wqk")
            wkv_sb = cpool.tile([128, NEC, 128], BF16, tag="wkv")
            w2d_sb = cpool.tile([128, NLT, 128], BF16, tag="w2d")
            cosT_sb = cpool.tile([128, L], BF16, tag="cosT")
            sinT_sb = cpool.tile([128, L], BF16, tag="sinT")
            cosN_sb = cpool.tile([C, NCH], BF16, tag="cosN")
            sinN_sb = cpool.tile([C, NCH], BF16, tag="sinN")
            mask_sb = cpool.tile([C, C], BF16, tag="mask")
            ident_sb = cpool.tile([C, C], BF16, tag="ident")

            for t_sb, t_d in [(wqk_sb, wqk), (wkv_sb, wkv)]:
                nc.sync.dma_start(
                    t_sb[:, :4, :],
                    t_d[: 4 * 128, :].rearrange("(c p) d -> p c d", p=128),
                )
                if NEC == 5:
                    nc.sync.dma_start(t_sb[:1, 4, :], t_d[4 * 128 :, :])
            nc.sync.dma_start(
                cosT_sb[:], cosTd[:1, :].to_broadcast([128, L]))
            nc.sync.dma_start(
                sinT_sb[:], sinTd[:1, :].to_broadcast([128, L]))
            nc.sync.dma_start(
                w2d_sb[:], w2d.rearrange("d (t n) -> d t n", n=128)
            )
            nc.sync.dma_start(cosN_sb[:], cosNd[:])
            nc.sync.dma_start(sinN_sb[:], sinNd[:])
            nc.sync.dma_start(mask_sb[:], maskd[:])
            nc.sync.dma_start(ident_sb[:], identd[:])

            def make_tiles():
                tl = {}
                tl["xT"] = wpool.tile([128, NEC, L], BF16, tag="xT", name="xTs")
                tl["qk"] = wpool.tile([128, L], BF16, tag="qk", name="qkt")
                tl["qcqs"] = wpool.tile([128, L], BF16, tag="qcqs", name="qcqst")
                tl["qstmp"] = wpool.tile([D, L], BF16, tag="qstmp", name="qstmpt")
                tl["kvt"] = wpool.tile([128, L], BF16, tag="kvt", name="kvtt")
                tl["kn"] = wpool.tile([C, NCH, 128], BF16, tag="kn", name="knt")
                tl["ktv"] = wpool.tile([C, NCH, 129], BF16, tag="ktv", name="ktvt")
                tl["at"] = wpool.tile([C, NCH, C], BF16, tag="at", name="att")
                tl["kvs"] = wpool.tile([128, NCH - 1, D + 1], BF16, tag="kvs", name="kvst")
                if state2:
                    tl["kvsB"] = wpool.tile([128, 7, D + 1], BF16,
                                            tag="kvsB", name="kvsBt")
                tl["obf"] = wpool.tile([C, NCH, D], BF16, tag="obf", name="obft")
                tl["otb"] = wpool.tile([128, NCH // 2, C], BF16, tag="otb", name="otbt")
                tl["rtmp"] = wpool.tile([C, NCH], F32, tag="rtmp", name="rtmpt")
                tl["r"] = wpool.tile([C, NCH], F32, tag="r", name="rt")
                return tl

            def a_parts(tl):
                """Input DMA + projections + scalings for one body, as a list
                of emission thunks (interleaved into the previous body)."""
                parts = []

                def dma_x(eng):
                    def f():
                        if "xdma" in abl:
                            nc.gpsimd.memset(
                                tl["ktv"][:, :, 128:129], 1.0)
                            return
                        if xsplit:
                            nc.sync.dma_start(
                                tl["xT"][:, :2, :],
                                xT[:, : 2 * L].rearrange(
                                    "p (c l) -> p c l", l=L),
                            )
                            nc.scalar.dma_start(
                                tl["xT"][:, 2:4, :],
                                xT[:, 2 * L :].rearrange(
                                    "p (c l) -> p c l", l=L),
                            )
                        else:
                            eng.dma_start(
                                tl["xT"][:, :4, :],
                                xT[:, :].rearrange("p (c l) -> p c l", l=L),
                            )
                        if NEC == 5:
                            eng.dma_start(tl["xT"][:1, 4, :], xTb[:, :])
                        nc.gpsimd.memset(tl["ktv"][:, :, 128:129], 1.0)
                    return f

                def tproj(tp):
                    def f():
                        if "tstream" in abl:
                            return
                        pT = pbig.tile([128, 2, LT], F32, tag="big",
                                       name="pT")
                        for half in range(2):
                            t = 2 * tp + half
                            ls = slice(t * LT, (t + 1) * LT)
                            for i, (ec, pc) in enumerate(ecs):
                                nc.tensor.matmul(
                                    pT[:, half, :], wqk_sb[:pc, ec, :],
                                    tl["xT"][:pc, ec, ls],
                                    start=(i == 0), stop=(i == NEC - 1),
                                )
                        ls2 = slice(tp * 2 * LT, (tp + 1) * 2 * LT)
                        if tp == 0:
                            nc.scalar.activation(tl["qk"][:, ls2], pT[:],
                                                 AF.Relu)
                        else:
                            nc.vector.tensor_scalar_max(tl["qk"][:, ls2],
                                                        pT[:], 0.0)
                    return f

                def scal(hl):
                    def f():
                        if "scal" in abl:
                            return
                        ls = slice(hl * (L // 2), (hl + 1) * (L // 2))
                        qeng = nc.gpsimd if balance == "v2" else nc.vector
                        qeng.tensor_tensor(
                            tl["qcqs"][:D, ls], tl["qk"][:D, ls],
                            cosT_sb[:D, ls], ALU.mult,
                        )
                        qeng.tensor_tensor(
                            tl["qstmp"][:, ls], tl["qk"][:D, ls],
                            sinT_sb[:D, ls], ALU.mult,
                        )
                        if "shift" not in abl:
                            nc.scalar.dma_start(tl["qcqs"][D:, ls],
                                                tl["qstmp"][:, ls])
                    return f

                def kvtp(tp):
                    # second T-stream: kvt = [Wk|Wv]^T xT; k rows relu'd
                    def f():
                        if "nstream" in abl:
                            return
                        pT = pbig.tile([128, 2, LT], F32, tag="big",
                                       name="pKV")
                        for half in range(2):
                            t = 2 * tp + half
                            ls = slice(t * LT, (t + 1) * LT)
                            for i, (ec, pc) in enumerate(ecs):
                                nc.tensor.matmul(
                                    pT[:, half, :], wkv_sb[:pc, ec, :],
                                    tl["xT"][:pc, ec, ls],
                                    start=(i == 0), stop=(i == NEC - 1),
                                )
                        ls2 = slice(tp * 2 * LT, (tp + 1) * 2 * LT)
                        nc.scalar.activation(tl["kvt"][:D, ls2],
                                             pT[:D, :, :], AF.Relu)
                        nc.vector.tensor_copy(tl["kvt"][D:, ls2],
                                              pT[D:, :, :])
                    return f

                def ktr(h):
                    # paired transposes: chunk-row layout ktv[l, j, [k'|v]]
                    def f():
                        if "nstream" in abl:
                            return
                        gs = slice(h * 2 * GRP, (h + 1) * 2 * GRP)
                        ptr = pso.tile([C, 2 * GRP, 128], BF16, tag="so",
                                       name="ptr")
                        for sub in range(2 * GRP):
                            j = h * 2 * GRP + sub
                            cs = slice(j * C, (j + 1) * C)
                            nc.tensor.transpose(
                                ptr[:, sub, :], tl["kvt"][:, cs],
                                ident_sb[:],
                            )
                        if h == 0:
                            nc.scalar.activation(tl["ktv"][:, gs, :128],
                                                 ptr[:], AF.Copy)
                        else:
                            nc.vector.tensor_copy(tl["ktv"][:, gs, :128],
                                                  ptr[:])
                        nc.gpsimd.tensor_tensor(
                            tl["kn"][:, gs, :D], tl["ktv"][:, gs, :D],
                            cosN_sb[:, gs, None].to_broadcast(
                                [C, 2 * GRP, D]),
                            ALU.mult,
                        )
                        nc.gpsimd.tensor_tensor(
                            tl["kn"][:, gs, D:], tl["ktv"][:, gs, :D],
                            sinN_sb[:, gs, None].to_broadcast(
                                [C, 2 * GRP, D]),
                            ALU.mult,
                        )
                    return f

                # HWDGE engines only (sync/scalar): a gpsimd (SWDGE) DMA in
                # a For_i body makes the loop reset emit INC_SWDGE_SEM,
                # which this container's walrus cannot encode.
                parts.append(dma_x(nc.sync))
                parts.append(tproj(0))
                parts.append(scal(0))
                parts.append(tproj(1))
                parts.append(scal(1))
                parts.append(kvtp(0))
                parts.append(ktr(0))
                parts.append(kvtp(1))
                parts.append(ktr(1))
                return parts

            def state_steps(tl, kv_ps, g):
                # kv accumulation + snapshots for chunks of group g (emitted
                # ahead of the group's O work; chain overlaps S0/intra phase)
                if "state" in abl:
                    return
                for sub in range(GRP):
                    j = g * GRP + sub
                    if j < NCH - 1:
                        nc.tensor.matmul(
                            kv_ps[:, : D + 1], tl["kn"][:, j, :],
                            tl["ktv"][:, j, D:],
                            start=(j == 0), stop=(j == NCH - 2),
                            skip_group_check=True,
                        )
                        if j % 2 == 0:
                            nc.scalar.activation(
                                tl["kvs"][:, j, :], kv_ps[:, : D + 1],
                                AF.Copy)
                        else:
                            nc.vector.tensor_copy(
                                tl["kvs"][:, j, :], kv_ps[:, : D + 1])

            def state_steps2(tl, kv_psA, kv_psB, g):
                # two parallel prefix chains (chunks 0-7 / 8-14), both done
                # by end of group 1; halves the serial mm->snapshot latency.
                # kvs[j>=8] gets chain-A's total added in one batched DVE op
                # (one extra bf16 rounding vs the single-chain version).
                if "state" in abl or g >= 2:
                    return
                for st in range(GRP):
                    jA = g * GRP + st
                    nc.tensor.matmul(
                        kv_psA[:, : D + 1], tl["kn"][:, jA, :],
                        tl["ktv"][:, jA, D:],
                        start=(jA == 0), stop=(jA == 7),
                        skip_group_check=True,
                    )
                    nc.scalar.activation(
                        tl["kvs"][:, jA, :], kv_psA[:, : D + 1], AF.Copy)
                    jB = 8 + g * GRP + st
                    if jB <= 14:
                        nc.tensor.matmul(
                            kv_psB[:, : D + 1], tl["kn"][:, jB, :],
                            tl["ktv"][:, jB, D:],
                            start=(jB == 8), stop=(jB == 14),
                            skip_group_check=True,
                        )
                        nc.vector.tensor_copy(
                            tl["kvsB"][:, jB - 8, :], kv_psB[:, : D + 1])
                if g == 1:
                    nc.vector.tensor_tensor(
                        tl["kvs"][:, 8:15, :], tl["kvsB"][:],
                        tl["kvs"][:, 7, None, :].to_broadcast(
                            [128, 7, D + 1]),
                        ALU.add,
                    )

            def b_group(tl, g):
                gs = slice(g * GRP, (g + 1) * GRP)
                if "s0" not in abl:
                    b_group_s0(tl, g, gs)
                if "ov" not in abl:
                    b_group_ov(tl, g, gs)

            def b_group_s0(tl, g, gs):
                s0 = pso.tile([C, GRP, C], F32, tag="so")
                for sub in range(GRP):
                    j = g * GRP + sub
                    cs = slice(j * C, (j + 1) * C)
                    nc.tensor.matmul(
                        s0[:, sub, :], tl["kvt"][:D, cs], tl["qk"][:D, cs],
                        start=True, stop=True,
                    )
                nc.vector.tensor_tensor(
                    tl["at"][:, gs, :], s0[:],
                    mask_sb[:, None, :].to_broadcast([C, GRP, C]), ALU.mult,
                )

            def b_group_ov(tl, g, gs):
                o_ps = pso.tile([C, GRP, 128], F32, tag="so")
                for sub in range(GRP):
                    j = g * GRP + sub
                    cs = slice(j * C, (j + 1) * C)
                    nc.tensor.matmul(
                        o_ps[:, sub, : D + 1], tl["at"][:, j, :],
                        tl["ktv"][:, j, D:],
                        start=True, stop=(j == 0),
                    )
                    if j > 0:
                        nc.tensor.matmul(
                            o_ps[:, sub, : D + 1], tl["qcqs"][:, cs],
                            tl["kvs"][:, j - 1, :],
                            start=False, stop=True,
                        )
                if "norm" in abl:
                    nc.scalar.activation(tl["obf"][:, gs, :],
                                         o_ps[:, :, :D], AF.Copy)
                    return
                nc.vector.tensor_scalar_add(
                    tl["rtmp"][:, gs], o_ps[:, :, D], EPS)
                nc.vector.reciprocal(tl["r"][:, gs], tl["rtmp"][:, gs])
                nc.vector.tensor_tensor(
                    tl["obf"][:, gs, :], o_ps[:, :, :D],
                    tl["r"][:, gs, None].to_broadcast([C, GRP, D]), ALU.mult,
                )

            def c_tr(tl, h2):
                if "ctr" in abl:
                    return
                tp = pso.tile([128, NCH // 4, C], BF16, tag="so", name="tp")
                for tt in range(NCH // 4):
                    t = h2 * (NCH // 4) + tt
                    nc.tensor.transpose(
                        tp[:, tt, :], tl["obf"][:, 2 * t : 2 * t + 2, :],
                        ident_sb[:],
                    )
                ts = slice(h2 * 4, (h2 + 1) * 4)
                if h2 == 0 or balance == "v1":
                    nc.vector.tensor_copy(tl["otb"][:, ts, :], tp[:])
                else:
                    nc.scalar.activation(tl["otb"][:, ts, :], tp[:], AF.Copy)

            def c_p3(tl, h2, nss):
                if "oproj" in abl:
                    return
                ts = slice(h2 * 4, (h2 + 1) * 4)
                ob = spool.tile([128, 4, 2, LT], BF16, tag="ob")
                for ns in nss:
                    pab = pbig.tile([128, 2, LT], F32, tag="big", name="pab")
                    tp0 = (0, 0) if p3tile else None
                    tp1 = (64, 0) if p3tile else None
                    nc.tensor.matmul(
                        pab[:, 0, :], w2d_sb[:D, ns, :], tl["otb"][:D, ts, :],
                        start=True, stop=True, tile_position=tp0,
                    )
                    nc.tensor.matmul(
                        pab[:, 1, :], w2d_sb[D:, ns, :], tl["otb"][D:, ts, :],
                        start=True, stop=True, tile_position=tp1,
                    )
                    dve = (ns % 2 == 1) if balance == "v2" \
                        else ((h2 * NLT + ns) % 4 == 1)
                    if dve:
                        nc.vector.tensor_copy(ob[:, ns], pab[:])
                    else:
                        nc.scalar.activation(ob[:, ns], pab[:], AF.Copy)
                if "store" not in abl:
                    if store_eng == "split":
                        seng = nc.scalar if h2 == 0 else nc.sync
                    else:
                        seng = nc.sync if store_eng == "sync" else nc.scalar
                    seng.dma_start(
                        outP[:, h2, :, :],
                        ob[:].rearrange("p n a b -> p n (a b)"),
                    )

            def one_body(tl, pending=()):
                """Emit one body's compute, interleaving `pending` emission
                thunks (the next body's input DMA/projections) into the
                group structure so in-order engines fill dependency stalls."""
                pi = iter(pending)

                def drain(n):
                    for _ in range(n):
                        p = next(pi, None)
                        if p is not None:
                            p()

                if state2:
                    kv_psA = pkv.tile([128, D + 1], F32, tag="kv")
                    kv_psB = pkv.tile([128, D + 1], F32, tag="kv")
                else:
                    kv_ps = pkv.tile([128, D + 1], F32, tag="kv")
                for g in range(NG):
                    if state2:
                        state_steps2(tl, kv_psA, kv_psB, g)
                    else:
                        state_steps(tl, kv_ps, g)
                    b_group(tl, g)
                    if g == 1:
                        c_tr(tl, 0)
                        if not p3wide:
                            c_p3(tl, 0, [0, 1, 2, 3])
                    drain(2)
                c_tr(tl, 1)
                if p3wide:
                    c_p3w(tl)
                else:
                    c_p3(tl, 1, [0, 1, 2, 3])
                for p in pi:
                    p()

            if hw_loop:
                if repeat == 1:
                    cur = make_tiles()
                    for p in a_parts(cur):
                        p()
                    one_body(cur)
                    return nc
                assert repeat % unroll == 0, (
                    "hw_loop repeat must be 1 or a multiple of unroll")
                tA = make_tiles()
                tB = make_tiles()
                for p in a_parts(tA):
                    p()
                with tc.For_i(0, repeat // unroll):
                    for u in range(unroll // 2):
                        one_body(tA, a_parts(tB))
                        one_body(tB, a_parts(tA))
                return nc

            def c_p3w(tl):
                # out-proj as 8 single-shot N=1024 matmuls (bf16 psum out);
                # slot order: 0-7 = even chunks (otb top), 8-15 = odd (bottom)
                if "oproj" in abl:
                    return
                obs = []
                for half in range(2):
                    ob = spool.tile([128, 4, 2, LT], BF16, tag="ob")
                    hs = slice(0, D) if half == 0 else slice(D, 128)
                    for ns in range(4):
                        pab = pbig.tile([128, 2 * GRP, C], F32, tag="big",
                                        name="pabw")
                        nc.tensor.matmul(
                            pab[:], w2d_sb[hs, ns, :], tl["otb"][hs, :, :],
                            start=True, stop=True,
                        )
                        if (2 * half + ns) % 2 == 0:
                            nc.scalar.activation(
                                ob[:, ns].rearrange("p a b -> p (a b)"),
                                pab[:].rearrange("p s c -> p (s c)"),
                                AF.Copy)
                        else:
                            nc.vector.tensor_copy(
                                ob[:, ns].rearrange("p a b -> p (a b)"),
                                pab[:].rearrange("p s c -> p (s c)"))
                    obs.append(ob)
                for half in range(2):
                    seng = nc.sync if store_eng == "sync" else nc.scalar
                    seng.dma_start(
                        outP[:, half, :, :],
                        obs[half][:].rearrange("p n a b -> p n (a b)"),
                    )

            # ---- software-pipelined emission across unrolled bodies ----
            cur = make_tiles()
            for p in a_parts(cur):
                p()
            for rep in range(repeat):
                if rep + 1 < repeat:
                    nxt = make_tiles()
                    pending = a_parts(nxt)
                else:
                    nxt, pending = None, []
                one_body(cur, pending)
                cur = nxt

    return nc


def prepare_in_maps(x, W_qkv, b_qkv, W_out):
    """Host-side sharding/layout prep. Returns (in_maps, e_in)."""
    x = np.asarray(x, dtype=np.float32).reshape(L, E)
    W_qkv = np.asarray(W_qkv, dtype=np.float32)
    b_qkv = np.asarray(b_qkv, dtype=np.float32)
    W_out = np.asarray(W_out, dtype=np.float32)

    use_bias = bool(np.any(b_qkv))
    if use_bias:
        x_aug = np.concatenate([x, np.ones((L, 1), np.float32)], axis=1)
        W_aug = np.concatenate([W_qkv, b_qkv[None, :]], axis=0)
    else:
        x_aug, W_aug = x, W_qkv
    e_in = x_aug.shape[1]

    bf = ml_dtypes.bfloat16
    # partition-major: xT[p, c*L + l] = x_aug[l, c*128 + p]
    xTf = np.ascontiguousarray(x_aug.T).astype(bf)          # [e_in, L]
    xT = np.ascontiguousarray(
        xTf[: 4 * 128].reshape(4, 128, L).transpose(1, 0, 2).reshape(
            128, 4 * L))
    xTb = np.ascontiguousarray(xTf[4 * 128 :]) if e_in > 4 * 128 else None

    pos = np.arange(L, dtype=np.float32)
    theta = (np.pi / 2) * pos / L
    cosw = np.cos(theta).astype(np.float32)
    sinw = np.sin(theta).astype(np.float32)
    cosT = np.ascontiguousarray(cosw[None, :]).astype(bf)
    sinT = np.ascontiguousarray(sinw[None, :]).astype(bf)
    cosN = np.ascontiguousarray(cosw.reshape(NCH, C).T).astype(bf)
    sinN = np.ascontiguousarray(sinw.reshape(NCH, C).T).astype(bf)
    # mask[m, l] = cos(theta_l - theta_m) for m <= l else 0  (Toeplitz intra)
    lm = np.arange(C, dtype=np.float32)
    dth = (np.pi / 2) * (lm[None, :] - lm[:, None]) / L
    mask = (np.triu(np.ones((C, C), np.float32)) * np.cos(dth)).astype(bf)
    ident = np.eye(C, dtype=np.float32).astype(bf)

    in_maps = []
    for h in range(N_CORES):
        hs = slice(h * D, (h + 1) * D)
        wq_h = W_aug[:, hs]
        wk_h = W_aug[:, E + h * D : E + (h + 1) * D]
        wv_h = W_aug[:, 2 * E + h * D : 2 * E + (h + 1) * D]
        wqk_h = np.ascontiguousarray(
            np.concatenate([wq_h, wk_h], axis=1)).astype(bf)
        wkv_h = np.ascontiguousarray(
            np.concatenate([wk_h, wv_h], axis=1)).astype(bf)
        w2_h = W_out[hs, :]
        w2d_h = np.ascontiguousarray(
            np.concatenate([w2_h, w2_h], axis=0)).astype(bf)
        im = {
            "xT": xT, "wqk": wqk_h, "wkv": wkv_h, "w2d": w2d_h,
            "cosT": cosT, "sinT": sinT, "cosN": cosN, "sinN": sinN,
            "mask": mask, "ident": ident,
        }
        if xTb is not None:
            im["xTb"] = xTb
        in_maps.append(im)
    return in_maps, e_in


def combine_outputs(results, b_out, p3wide=None):
    if p3wide is None:
        p3wide = P3WIDE
    slot_order = CHUNK_OF_SLOT_WIDE if p3wide else CHUNK_OF_SLOT
    b_out = np.asarray(b_out, dtype=np.float32)
    accP = np.zeros((128, 2, 4, 1024), np.float32)
    for r in results:
        accP += np.asarray(r["outP"]).astype(np.float32)
    # outP[p, h2, n, l'] -> acc[n*128+p, h2*1024+l']
    acc = np.ascontiguousarray(
        accP.transpose(2, 0, 1, 3).reshape(E, L))
    # unpermute column slots -> chunk order
    out = np.empty((E, L), np.float32)
    for s, ch in enumerate(slot_order):
        out[:, ch * C : (ch + 1) * C] = acc[:, s * C : (s + 1) * C]
    out = out.T + b_out[None, :]
    return out.reshape(B, L, E).astype(np.float32)


_PROGRAM_CACHE = {}


def _get_program(e_in):
    if e_in not in _PROGRAM_CACHE:
        _PROGRAM_CACHE[e_in] = build_program(e_in=e_in)
    return _PROGRAM_CACHE[e_in]


def kernel(x, W_qkv, b_qkv, W_out, b_out):
    from concourse.bass_utils import run_bass_kernel_spmd

    in_maps, e_in = prepare_in_maps(x, W_qkv, b_qkv, W_out)
    nc = _get_program(e_in)
    res = run_bass_kernel_spmd(nc, in_maps, core_ids=list(range(N_CORES)))
    return combine_outputs(res.results, b_out)

